# revision 1
# baseline (speedup 1.0000x reference)
"""Trainium2 Bass kernel for nn_GCN4Rec (2-layer GCNConv + user/item dot).

Sharding (8 NeuronCores):
  - item/node space padded to 100352 rows; each core owns 98 blocks of 128
    nodes (blocks assigned to cores by a greedy edge-count balancer).
  - Graph edge-partitioned by destination block; self-loops are materialized
    as explicit edges.  Per (core, src-chunk, slot) the edges are sorted by
    source row and padded to 128-edge tiles (src-chunks of 32768 rows keep
    gather indices within int16 for dma_gather).
  - GCN math is reformulated so aggregation is a pure segment-sum of
    pre-scaled features:  out[n] = dinv[n] * sum_{e->n} xs[src_e]  (+bias),
    with  xs = max_norm(item_emb) * dinv  and the Linear layers applied per
    destination block after aggregation.  The segment-sum runs on the tensor
    engine as one-hot matmuls accumulating in PSUM (then SBUF across chunks);
    per-edge rows are fetched with bulk dma_gather (Q7 SWDGE ucode).
  - Feature tables (xs / z2 / x_out) are fp16 rows padded to 128 elements
    (256 B — dma_gather granularity) and AllGathered between layers.
  - users_emb is row-sharded (125k rows/core); the B=16384 pairs are bucketed
    by owning core of u; items are gathered from the AllGathered x_out.

kernel(**inputs) takes the full unsharded inputs and returns the full [B]
sigmoid logits, matching reference.py.
"""
import os
import numpy as np

import concourse.bass as bass
import concourse.bacc as bacc
import concourse.mybir as mybir
import concourse.tile as tile
from concourse.bass_utils import run_bass_kernel_spmd

# ---------------------------------------------------------------- constants
N_USERS = 1_000_000
N_ITEMS = 100_000
DIM = 64
HID = 128
B_PAIRS = 16_384

N_CORES = 8
P = 128
NB = 98                               # blocks (slots) per core
NODES_PAD = N_CORES * NB * P          # 100352
NODES_PER_CORE = NB * P               # 12544
USERS_PER_CORE = N_USERS // N_CORES   # 125000
ROWP = 128                            # padded feature-row length (256 B fp16)

CHUNK = 32768                         # int16 index range per gather chunk
NCHUNK = -(-NODES_PAD // CHUNK)       # 4

GATHER_TILES = int(os.environ.get("GCN_GT", "16"))   # tiles per dma_gather call
MSG_BUFS = int(os.environ.get("GCN_MSGBUFS", "2"))
STAGE = int(os.environ.get("GCN_STAGE", "4"))

f32 = mybir.dt.float32
f16 = mybir.dt.float16
i32 = mybir.dt.int32
i16 = mybir.dt.int16
AF = mybir.ActivationFunctionType
ALU = mybir.AluOpType

PREP_PASSES = 7                       # xs prep processed 14 slots at a time
PREP_SLOTS = NB // PREP_PASSES        # 14


# ---------------------------------------------------------------- host layout
def _build_layout(u, i, edges):
    """Pure index/layout computation (the sharding step)."""
    src = np.asarray(edges[0], dtype=np.int64)
    dst = np.asarray(edges[1], dtype=np.int64)

    selfn = np.arange(N_ITEMS, dtype=np.int64)
    src_a = np.concatenate([src, selfn])
    dst_a = np.concatenate([dst, selfn])

    deg_raw = np.bincount(dst, minlength=NODES_PAD).astype(np.int32)

    n_blocks = NODES_PAD // P
    blk = dst_a // P
    blk_cnt = np.bincount(blk, minlength=n_blocks)

    # greedy balance: largest blocks first onto least-loaded core
    order = np.argsort(-blk_cnt, kind="stable")
    core_tiles = np.zeros(N_CORES, dtype=np.int64)
    core_blocks = [[] for _ in range(N_CORES)]
    for b in order:
        cand = [c for c in range(N_CORES) if len(core_blocks[c]) < NB]
        k = min(cand, key=lambda c: core_tiles[c])
        core_blocks[k].append(b)
        core_tiles[k] += -(-blk_cnt[b] // P)
    assign = np.empty((n_blocks, 2), dtype=np.int64)
    for k in range(N_CORES):
        for s, b in enumerate(core_blocks[k]):
            assign[b] = (k, s)
    core_of_blk, slot_of_blk = assign[:, 0], assign[:, 1]

    nodes = np.arange(NODES_PAD, dtype=np.int64)
    pos_of_node = (core_of_blk[nodes // P] * NODES_PER_CORE
                   + slot_of_blk[nodes // P] * P + nodes % P)
    node_of_pos = np.empty(NODES_PAD, dtype=np.int64)
    node_of_pos[pos_of_node] = nodes

    # edge keys: (core, chunk, slot, src)
    ecore = core_of_blk[blk]
    eslot = slot_of_blk[blk]
    spos = pos_of_node[src_a]
    echunk = spos // CHUNK
    seg = (ecore * NCHUNK + echunk) * NB + eslot
    okey = seg * (NODES_PAD + 1) + spos
    eorder = np.argsort(okey, kind="stable")
    seg_s = seg[eorder]
    spos_s = spos[eorder]
    dloc_s = (dst_a[eorder] % P).astype(np.float16)

    nseg = N_CORES * NCHUNK * NB
    seg_start = np.searchsorted(seg_s, np.arange(nseg))
    seg_end = np.searchsorted(seg_s, np.arange(nseg) + 1)
    cnt_kcs = (seg_end - seg_start).reshape(N_CORES, NCHUNK, NB)
    tiles_kcs = -(-cnt_kcs // P)
    T_cs = tiles_kcs.max(axis=0)           # [NCHUNK, NB] compile-time
    TOT = int(T_cs.sum())

    # tile offsets per (chunk, slot), chunk-major
    off_cs = np.zeros((NCHUNK, NB), dtype=np.int64)
    a = 0
    for c in range(NCHUNK):
        for s in range(NB):
            off_cs[c, s] = a
            a += int(T_cs[c, s])

    gidx = np.zeros((N_CORES, 16, TOT * 8), dtype=np.int16)
    dstloc = np.full((N_CORES, P, TOT), -1.0, dtype=np.float16)
    for k in range(N_CORES):
        for c in range(NCHUNK):
            for s in range(NB):
                sidx = (k * NCHUNK + c) * NB + s
                a, b_ = seg_start[sidx], seg_end[sidx]
                n = b_ - a
                if n == 0:
                    continue
                j0 = off_cs[c, s] * P
                jj = j0 + np.arange(n)
                gidx[k, jj % 16, jj // 16] = (spos_s[a:b_] - c * CHUNK)
                dstloc[k, jj % P, jj // P] = dloc_s[a:b_]
    gidx = np.tile(gidx, (1, 8, 1))         # replicate to 128 partitions

    deg_cs = np.zeros((N_CORES, P, NB), dtype=np.int32)
    perm_nodes = node_of_pos.reshape(N_CORES, NB, P)
    for k in range(N_CORES):
        deg_cs[k] = deg_raw[perm_nodes[k]].T

    # final-stage buckets by user-owner core
    u = np.asarray(u, dtype=np.int64)
    i = np.asarray(i, dtype=np.int64)
    owner = u // USERS_PER_CORE
    border = np.argsort(owner, kind="stable")
    bstart = np.searchsorted(owner[border], np.arange(N_CORES))
    bend = np.searchsorted(owner[border], np.arange(N_CORES) + 1)
    bmax = int((bend - bstart).max())
    BT = max(1, -(-bmax // P))
    u_loc = np.zeros((N_CORES, P, BT), dtype=np.int32)
    i_pos = np.zeros((N_CORES, P, BT), dtype=np.int32)
    perm = []
    for k in range(N_CORES):
        sel = border[bstart[k]:bend[k]]
        perm.append(sel)
        jj = np.arange(len(sel))
        u_loc[k, jj % P, jj // P] = u[sel] - k * USERS_PER_CORE
        i_pos[k, jj % P, jj // P] = pos_of_node[i[sel]]

    return dict(T_cs=T_cs, off_cs=off_cs, TOT=TOT, BT=BT,
                gidx=gidx, dstloc=dstloc, deg_cs=deg_cs,
                u_loc=u_loc, i_pos=i_pos, perm=perm,
                node_of_pos=node_of_pos)


# ---------------------------------------------------------------- bass build
def _build_bass(T_cs, TOT, BT):
    nc = bacc.Bacc("TRN2", target_bir_lowering=False, debug=False,
                   num_devices=N_CORES)

    item_part = nc.dram_tensor("item_part", [NODES_PER_CORE, DIM], f32,
                               kind="ExternalInput")
    users_part = nc.dram_tensor("users_part", [USERS_PER_CORE, DIM], f32,
                                kind="ExternalInput")
    deg_in = nc.dram_tensor("deg", [P, NB], i32, kind="ExternalInput")
    gidx_in = nc.dram_tensor("gidx", [P, TOT * 8], i16, kind="ExternalInput")
    dstloc_in = nc.dram_tensor("dstloc", [P, TOT], f16, kind="ExternalInput")
    w1_in = nc.dram_tensor("w1", [DIM, HID], f16, kind="ExternalInput")
    w2_in = nc.dram_tensor("w2", [HID, DIM], f16, kind="ExternalInput")
    b1_in = nc.dram_tensor("b1", [P, 1], f32, kind="ExternalInput")
    b2b_in = nc.dram_tensor("b2b", [P, DIM], f32, kind="ExternalInput")
    iota_in = nc.dram_tensor("iotaf", [P, P], f16, kind="ExternalInput")
    ident_in = nc.dram_tensor("identf", [P, P], f16, kind="ExternalInput")
    uloc_in = nc.dram_tensor("u_loc", [P, BT], i32, kind="ExternalInput")
    ipos_in = nc.dram_tensor("i_pos", [P, BT], i32, kind="ExternalInput")
    out_t = nc.dram_tensor("out", [P, BT], f32, kind="ExternalOutput")

    rg = [list(range(N_CORES))]

    # tile offsets per (chunk, slot)
    off_cs = {}
    a = 0
    for c in range(NCHUNK):
        for s in range(NB):
            off_cs[(c, s)] = a
            a += int(T_cs[c, s])

    # gather-call groups: runs of tiles within one chunk, <= GATHER_TILES,
    # aligned to (chunk, slot) segment boundaries
    groups = []
    for c in range(NCHUNK):
        run0, rl = None, 0
        for s in range(NB):
            T = int(T_cs[c, s])
            if T == 0:
                continue
            if run0 is None:
                run0, rl = off_cs[(c, s)], 0
            if rl and rl + T > GATHER_TILES:
                groups.append((c, run0, rl))
                run0, rl = off_cs[(c, s)], 0
            rl += T
        if rl:
            groups.append((c, run0, rl))
    gmax = max(n for _, _, n in groups)

    with tile.TileContext(nc) as tc:
        with (
            tc.tile_pool(name="const", bufs=1) as cpool,
            tc.tile_pool(name="prep", bufs=2) as ppool,
            tc.tile_pool(name="acc", bufs=1) as accpool,
            tc.tile_pool(name="msg", bufs=MSG_BUFS) as msgpool,
            tc.tile_pool(name="oneh", bufs=3) as spool,
            tc.tile_pool(name="flush", bufs=3) as fpool,
            tc.tile_pool(name="pag", bufs=3, space="PSUM") as pagg,
            tc.tile_pool(name="pfa", bufs=2, space="PSUM") as pfa,
            tc.tile_pool(name="pfb", bufs=2, space="PSUM") as pfb,
            tc.tile_pool(name="dram", bufs=1, space="DRAM") as dpool,
        ):
            # ---------- constants ----------
            iota_sb = cpool.tile([P, P], f16)
            nc.sync.dma_start(iota_sb[:], iota_in[:, :])
            ident_sb = cpool.tile([P, P], f16)
            nc.sync.dma_start(ident_sb[:], ident_in[:, :])
            w1_sb = cpool.tile([DIM, HID], f16)
            nc.sync.dma_start(w1_sb[:], w1_in[:, :])
            w2_sb = cpool.tile([HID, DIM], f16)
            nc.sync.dma_start(w2_sb[:], w2_in[:, :])
            b1_sb = cpool.tile([P, 1], f32)
            nc.sync.dma_start(b1_sb[:], b1_in[:, :])
            b2b_sb = cpool.tile([P, DIM], f32)
            nc.sync.dma_start(b2b_sb[:], b2b_in[:, :])

            gidx_sb = cpool.tile([P, TOT * 8], i16)
            nc.sync.dma_start(gidx_sb[:], gidx_in[:, :])
            dstloc_sb = cpool.tile([P, TOT], f16)
            nc.sync.dma_start(dstloc_sb[:], dstloc_in[:, :])

            # ---------- dinv ----------
            deg_sb = cpool.tile([P, NB], i32)
            nc.sync.dma_start(deg_sb[:], deg_in[:, :])
            degf = cpool.tile([P, NB], f32)
            nc.vector.tensor_copy(degf[:], deg_sb[:])
            dsq = cpool.tile([P, NB], f32)
            nc.scalar.activation(dsq[:], degf[:], AF.Sqrt, bias=1.0, scale=1.0)
            dinv_sb = cpool.tile([P, NB], f32)
            nc.vector.reciprocal(dinv_sb[:], dsq[:])

            # ---------- xs slice ----------
            acc = accpool.tile([P, NB * DIM], f16, tag="acc")
            for pp in range(PREP_PASSES):
                s0 = pp * PREP_SLOTS
                ns = PREP_SLOTS
                item_sb = ppool.tile([P, ns * DIM], f32, tag="item")
                nc.sync.dma_start(
                    item_sb[:].rearrange("p (j f) -> p j f", f=DIM),
                    item_part[:, :].rearrange("(j p) f -> p j f", p=P)
                    [:, s0:s0 + ns, :])
                sq = ppool.tile([P, ns * DIM], f32, tag="sq")
                nc.vector.tensor_tensor(sq[:], item_sb[:], item_sb[:],
                                        ALU.mult)
                ss = ppool.tile([P, ns], f32, tag="ss")
                nc.vector.tensor_reduce(
                    ss[:], sq[:].rearrange("p (j f) -> p j f", f=DIM),
                    axis=mybir.AxisListType.X, op=ALU.add)
                nrm = ppool.tile([P, ns], f32, tag="nrm")
                nc.scalar.activation(nrm[:], ss[:], AF.Sqrt)
                nrm1 = ppool.tile([P, ns], f32, tag="nrm1")
                nc.vector.tensor_scalar_max(nrm1[:], nrm[:], 1.0)
                rinv = ppool.tile([P, ns], f32, tag="rinv")
                nc.vector.reciprocal(rinv[:], nrm1[:])
                scal = ppool.tile([P, ns], f32, tag="scal")
                nc.vector.tensor_tensor(scal[:], rinv[:],
                                        dinv_sb[:, s0:s0 + ns], ALU.mult)
                # 2D first-column op absorbs waits; 3D op needs <=1 wait
                nc.vector.tensor_tensor(acc[:, s0 * DIM:s0 * DIM + 1],
                                        item_sb[:, 0:1], scal[:, 0:1],
                                        ALU.mult)
                nc.vector.tensor_tensor(
                    acc[:, s0 * DIM:(s0 + ns) * DIM].rearrange(
                        "p (j f) -> p j f", f=DIM),
                    item_sb[:].rearrange("p (j f) -> p j f", f=DIM),
                    scal[:, :, None].to_broadcast([P, ns, DIM]),
                    ALU.mult)

            xs_slice = dpool.tile([NODES_PER_CORE, ROWP], f16)
            nc.sync.dma_start(
                xs_slice[:].rearrange("(j p) F -> p j F", p=P)[:, :, 0:DIM],
                acc[:].rearrange("p (j f) -> p j f", f=DIM))
            xs_full = dpool.tile([NODES_PAD, ROWP], f16, addr_space="Shared")
            nc.gpsimd.collective_compute(
                "AllGather", ALU.bypass, replica_groups=rg,
                ins=[xs_slice.opt()], outs=[xs_full.opt()])

            aggall = accpool.tile([P, NB * DIM], f32, tag="aggall")

            def layer(table, is_first):
                first_c = {}
                for s in range(NB):
                    for c in range(NCHUNK):
                        if int(T_cs[c, s]) > 0:
                            first_c[s] = c
                            break

                msgs = {}
                for (c, t0, ntl) in groups:
                    m = msgpool.tile([P, gmax * ROWP], f16, tag="msg")
                    crows = min(CHUNK, NODES_PAD - c * CHUNK)
                    nc.gpsimd.dma_gather(
                        m[:, :ntl * ROWP].rearrange(
                            "p (t F) -> p t F", F=ROWP),
                        table[c * CHUNK:c * CHUNK + crows, :],
                        gidx_sb[:, t0 * 8:(t0 + ntl) * 8],
                        ntl * P, ntl * P, ROWP, single_packet=False)
                    msgs[t0] = m

                gi = 0
                for c in range(NCHUNK):
                    for s in range(NB):
                        T = int(T_cs[c, s])
                        if T == 0:
                            continue
                        st = off_cs[(c, s)]
                        while not (groups[gi][1] <= st
                                   < groups[gi][1] + groups[gi][2]):
                            gi += 1
                        gt0 = groups[gi][1]
                        m = msgs[gt0]
                        S = spool.tile([P, T * P], f16, tag="oneh")
                        nc.vector.tensor_tensor(
                            S[:, 0:P],
                            dstloc_sb[:, st:st + 1].to_broadcast([P, P]),
                            iota_sb[:], ALU.is_equal)
                        # overlapping 3D op keeps program order; <=1 wait
                        nc.vector.tensor_tensor(
                            S[:].rearrange("p (t n) -> p t n", n=P),
                            dstloc_sb[:, st:st + T].to_broadcast([P, T, P]),
                            iota_sb[:, None, :].to_broadcast([P, T, P]),
                            ALU.is_equal)
                        agg = pagg.tile([P, DIM], f32, tag="agg")
                        for t in range(T):
                            tl = st + t - gt0
                            nc.tensor.matmul(
                                agg[:],
                                lhsT=S[:, t * P:(t + 1) * P],
                                rhs=m[:].rearrange(
                                    "p (t F) -> p t F", F=ROWP)[:, tl, 0:DIM],
                                start=(t == 0), stop=(t == T - 1))
                        sl = slice(s * DIM, (s + 1) * DIM)
                        if first_c[s] == c:
                            nc.scalar.activation(aggall[:, sl], agg[:],
                                                 AF.Copy)
                        else:
                            nc.vector.tensor_tensor(aggall[:, sl],
                                                    aggall[:, sl], agg[:],
                                                    ALU.add)

                for s in range(NB):
                    sl = slice(s * DIM, (s + 1) * DIM)
                    if is_first:
                        aggsb = fpool.tile([P, DIM], f16, tag="aggsb")
                        nc.scalar.activation(aggsb[:], aggall[:, sl], AF.Copy,
                                             scale=dinv_sb[:, s:s + 1])
                        aggT = pfa.tile([DIM, P], f16, tag="pfa")
                        nc.tensor.transpose(aggT[:], aggsb[:], ident_sb[:])
                        aggT_sb = fpool.tile([DIM, P], f16, tag="aggT")
                        nc.scalar.activation(aggT_sb[:], aggT[:], AF.Copy)
                        U = pfb.tile([P, P], f32, tag="pfb")
                        nc.tensor.matmul(U[:], lhsT=w1_sb[:], rhs=aggT_sb[:],
                                         start=True, stop=True)
                        H = fpool.tile([P, P], f16, tag="H")
                        nc.scalar.activation(H[:], U[:], AF.Relu,
                                             bias=b1_sb[:, 0:1], scale=1.0)
                        z2p = pfa.tile([DIM, P], f32, tag="pfa")
                        nc.tensor.matmul(z2p[:], lhsT=w2_sb[:], rhs=H[:],
                                         start=True, stop=True)
                        z2p_sb = fpool.tile([DIM, P], f16, tag="z2p")
                        nc.scalar.activation(z2p_sb[:], z2p[:], AF.Copy)
                        z2r = pfb.tile([P, DIM], f16, tag="pfb")
                        nc.tensor.transpose(z2r[:], z2p_sb[:],
                                            ident_sb[:DIM, :DIM])
                        nc.scalar.activation(acc[:, sl], z2r[:], AF.Copy,
                                             scale=dinv_sb[:, s:s + 1])
                    else:
                        w = fpool.tile([P, DIM], f32, tag="wf32")
                        nc.scalar.activation(w[:], aggall[:, sl], AF.Copy,
                                             scale=dinv_sb[:, s:s + 1])
                        with nc.allow_low_precision(reason="fp16 out table"):
                            nc.vector.tensor_tensor(acc[:, sl], w[:],
                                                    b2b_sb[:], ALU.add)

            # ---------- layer 1 ----------
            if STAGE >= 2:
                layer(xs_full[:], True)
            z2_slice = dpool.tile([NODES_PER_CORE, ROWP], f16)
            nc.sync.dma_start(
                z2_slice[:].rearrange("(j p) F -> p j F", p=P)[:, :, 0:DIM],
                acc[:].rearrange("p (j f) -> p j f", f=DIM))
            z2_full = dpool.tile([NODES_PAD, ROWP], f16, addr_space="Shared")
            nc.gpsimd.collective_compute(
                "AllGather", ALU.bypass, replica_groups=rg,
                ins=[z2_slice.opt()], outs=[z2_full.opt()])

            # ---------- layer 2 ----------
            if STAGE >= 3:
                layer(z2_full[:], False)
            xo_slice = dpool.tile([NODES_PER_CORE, ROWP], f16)
            nc.sync.dma_start(
                xo_slice[:].rearrange("(j p) F -> p j F", p=P)[:, :, 0:DIM],
                acc[:].rearrange("p (j f) -> p j f", f=DIM))
            xo_full = dpool.tile([NODES_PAD, ROWP], f16, addr_space="Shared")
            nc.gpsimd.collective_compute(
                "AllGather", ALU.bypass, replica_groups=rg,
                ins=[xo_slice.opt()], outs=[xo_full.opt()])

            # ---------- final: users/items gather + dot + sigmoid ----------
            uloc_sb = cpool.tile([P, BT], i32)
            nc.sync.dma_start(uloc_sb[:], uloc_in[:, :])
            ipos_sb = cpool.tile([P, BT], i32)
            nc.sync.dma_start(ipos_sb[:], ipos_in[:, :])

            us = cpool.tile([P, BT * DIM], f32, tag="us")
            it = cpool.tile([P, BT * ROWP], f16, tag="it")
            nc.vector.memset(us[:], 0.01)
            nc.vector.memset(it[:], 0.01)
            for t in range(BT if STAGE >= 4 else 0):
                nc.gpsimd.indirect_dma_start(
                    out=us[:, t * DIM:(t + 1) * DIM],
                    out_offset=None, in_=users_part[:, :],
                    in_offset=bass.IndirectOffsetOnAxis(
                        ap=uloc_sb[:, t:t + 1], axis=0))
                nc.gpsimd.indirect_dma_start(
                    out=it[:, t * ROWP:(t + 1) * ROWP],
                    out_offset=None, in_=xo_full[:],
                    in_offset=bass.IndirectOffsetOnAxis(
                        ap=ipos_sb[:, t:t + 1], axis=0))

            it32 = cpool.tile([P, BT * DIM], f32, tag="it32")
            nc.vector.tensor_copy(
                it32[:].rearrange("p (t f) -> p t f", f=DIM),
                it[:].rearrange("p (t F) -> p t F", F=ROWP)[:, :, 0:DIM])
            pr = cpool.tile([P, BT * DIM], f32, tag="pr")
            nc.vector.tensor_tensor(pr[:], us[:], it32[:], ALU.mult)
            d = cpool.tile([P, BT], f32)
            nc.vector.tensor_reduce(
                d[:], pr[:].rearrange("p (t f) -> p t f", f=DIM),
                axis=mybir.AxisListType.X, op=ALU.add)

            usq = cpool.tile([P, BT * DIM], f32, tag="usq")
            nc.vector.tensor_tensor(usq[:], us[:], us[:], ALU.mult)
            ssu = cpool.tile([P, BT], f32)
            nc.vector.tensor_reduce(
                ssu[:], usq[:].rearrange("p (t f) -> p t f", f=DIM),
                axis=mybir.AxisListType.X, op=ALU.add)
            un = cpool.tile([P, BT], f32)
            nc.scalar.activation(un[:], ssu[:], AF.Sqrt)
            un1 = cpool.tile([P, BT], f32)
            nc.vector.tensor_scalar_max(un1[:], un[:], 1.0)
            ur = cpool.tile([P, BT], f32)
            nc.vector.reciprocal(ur[:], un1[:])

            d2 = cpool.tile([P, BT], f32)
            nc.vector.tensor_tensor(d2[:], d[:], ur[:], ALU.mult)
            sg = cpool.tile([P, BT], f32)
            nc.scalar.activation(sg[:], d2[:], AF.Sigmoid)
            nc.sync.dma_start(out_t[:, :], sg[:])

    nc.compile()
    return nc


def _legalize_waits(nc):
    """Split multi-sem waits off compute instructions onto preceding NoOps.

    walrus codegen's per-instruction ISA structs have a single sync-wait
    slot; Tile can attach several.  Standalone NoOps carry the extras (one
    wait per NoOp, executed in engine program order just before the
    instruction, so semantics are unchanged).
    """
    keep = (mybir.InstCollectiveCompute, mybir.InstNoOp,
            mybir.InstEventSemaphore, mybir.InstAllEngineBarrier)
    fn = nc.m.functions[0]
    n_split = 0
    for blk in fn.blocks:
        insts = list(blk.instructions)
        out = []
        changed = False
        for inst in insts:
            si = inst.sync_info
            if (si is not None and len(si.on_wait) > 1
                    and not isinstance(inst, keep)):
                waits = list(si.on_wait)
                for w_i, w in enumerate(waits[:-1]):
                    nop = mybir.InstNoOp(
                        name=f"wnop{w_i}_{inst.name}", ins=[], outs=[])
                    nop.engine = inst.engine
                    nop.sync_info = mybir.SyncInfo(on_wait=[w], on_update=[])
                    out.append(nop)
                inst.sync_info = mybir.SyncInfo(
                    on_wait=[waits[-1]], on_update=list(si.on_update))
                changed = True
                n_split += 1
            out.append(inst)
        if changed:
            blk.instructions = out
    return n_split


# ---------------------------------------------------------------- runner
_BUILD_CACHE = {}


def _get_bass(T_cs, TOT, BT):
    key = (tuple(int(t) for t in np.asarray(T_cs).ravel()), int(TOT), int(BT))
    if key not in _BUILD_CACHE:
        nc = _build_bass(np.asarray(T_cs), TOT, BT)
        _legalize_waits(nc)
        _BUILD_CACHE[key] = nc
    return _BUILD_CACHE[key]


def _make_in_maps(L, item_emb, users_emb, W1, b1, W2, b2):
    item_pad = np.zeros((NODES_PAD, DIM), np.float32)
    item_pad[:N_ITEMS] = np.asarray(item_emb, np.float32)
    item_perm = item_pad[L["node_of_pos"]]

    w1h = np.asarray(W1, np.float32).astype(np.float16)
    w2h = np.asarray(W2, np.float32).astype(np.float16)
    b1c = np.asarray(b1, np.float32).reshape(HID, 1)
    b2b = np.broadcast_to(np.asarray(b2, np.float32), (P, DIM)).copy()
    iotaf = np.broadcast_to(np.arange(P, dtype=np.float16), (P, P)).copy()
    identf = np.eye(P, dtype=np.float16)
    users = np.asarray(users_emb, np.float32)

    in_maps = []
    for k in range(N_CORES):
        in_maps.append({
            "item_part": item_perm[k * NODES_PER_CORE:(k + 1) * NODES_PER_CORE],
            "users_part": users[k * USERS_PER_CORE:(k + 1) * USERS_PER_CORE],
            "deg": L["deg_cs"][k],
            "gidx": L["gidx"][k],
            "dstloc": L["dstloc"][k],
            "w1": w1h, "w2": w2h, "b1": b1c, "b2b": b2b,
            "iotaf": iotaf, "identf": identf,
            "u_loc": L["u_loc"][k], "i_pos": L["i_pos"][k],
        })
    return in_maps


def _ensure_ntff_hook():
    """Register the axon NTFF profiling hook if the image lacks
    antenv.axon_hooks (needed for trace=True under axon)."""
    import sys
    import types
    try:
        from antenv.axon_hooks import get_axon_ntff_profile_hook  # noqa: F401
        return
    except ImportError:
        pass
    try:
        import antenv
        from trn_agent_boot.trn_boot import _ntff_profile_via_ctypes
        hook = _ntff_profile_via_ctypes("/opt/axon/libaxon_pjrt.so")
        mod = types.ModuleType("antenv.axon_hooks")
        box = {"v": hook}
        mod.set_axon_ntff_profile_hook = lambda h: box.__setitem__("v", h)
        mod.get_axon_ntff_profile_hook = lambda: box["v"]
        sys.modules["antenv.axon_hooks"] = mod
        antenv.axon_hooks = mod
    except Exception:
        pass


def run(u, i, edges, users_emb, item_emb, W1, b1, W2, b2, trace=False):
    if trace:
        _ensure_ntff_hook()
    L = _build_layout(u, i, edges)
    nc = _get_bass(L["T_cs"], L["TOT"], L["BT"])
    in_maps = _make_in_maps(L, item_emb, users_emb, W1, b1, W2, b2)
    res = run_bass_kernel_spmd(nc, in_maps, core_ids=list(range(N_CORES)),
                               trace=trace)
    logits = np.zeros(B_PAIRS, np.float32)
    for k in range(N_CORES):
        sel = L["perm"][k]
        o = np.asarray(res.results[k]["out"])
        logits[sel] = o.T.ravel()[:len(sel)]
    return logits, res


def kernel(u, i, edges, users_emb, item_emb, W1, b1, W2, b2):
    logits, _ = run(u, i, edges, users_emb, item_emb, W1, b1, W2, b2,
                    trace=bool(int(os.environ.get("GCN_TRACE", "0"))))
    return logits



# revision 18
# speedup vs baseline: 2.4727x; 2.4727x over previous
"""Trainium2 Bass kernel for nn_GCN4Rec (2-layer GCNConv + user/item dot).

Sharding (8 NeuronCores):
  - item/node space padded to 100352 rows; each core owns 98 blocks of 128
    nodes (blocks assigned to cores by a greedy layer-1 edge balancer; slot
    index = per-core size rank so per-(chunk,slot) counts align across cores).
  - Layer 1 is dst-partitioned: graph edges whose dst produces a needed z2
    row (sources of layer-2 edges) are bucketed per (core, src-chunk, slot),
    sorted by source row, and laid out on a shared row grid (max count over
    cores per cell, 1-row granularity).  Tiles of 128 rows may span slot
    boundaries; each (tile, slot) intersection is a "piece" with its own
    masked one-hot column (-1 rows contribute nothing).
  - Self-loops are never gathered: the aggregation accumulator is initialized
    from the local xs block already in SBUF.
  - Layer 2 is restricted to dst in unique(i) (the only rows the output
    reads) and src-partitioned: each core gathers from its LOCAL z2 slice
    (single int16 index chunk, no AllGather) and aggregates partial sums into
    a compact [~Ni, 64] table; partials are summed with one AllReduce.  The
    dst-side deg^{-1/2} and b2 are folded into the final per-pair dot.
  - GCN math is reformulated so aggregation is a pure segment-sum of
    pre-scaled features (xs = max_norm(item_emb) * dinv); the Linear layers
    are applied per destination block after aggregation.  The segment-sum
    runs on the tensor engine as one-hot matmuls accumulating in PSUM;
    per-edge rows are fetched with bulk dma_gather (Q7 SWDGE ucode).
  - users_emb is row-sharded (125k rows/core); the B=16384 pairs are bucketed
    by owning core of u; item vectors are gathered from the AllReduced
    compact table.

kernel(**inputs) takes the full unsharded inputs and returns the full [B]
sigmoid logits, matching reference.py.
"""
import os
import numpy as np

import concourse.bass as bass
import concourse.bacc as bacc
import concourse.mybir as mybir
import concourse.tile as tile
from concourse.bass_utils import run_bass_kernel_spmd

# ---------------------------------------------------------------- constants
N_USERS = 1_000_000
N_ITEMS = 100_000
DIM = 64
HID = 128
B_PAIRS = 16_384

N_CORES = 8
P = 128
NB = 98                               # blocks (slots) per core
NODES_PAD = N_CORES * NB * P          # 100352
NODES_PER_CORE = NB * P               # 12544
USERS_PER_CORE = N_USERS // N_CORES   # 125000
ROWP = 128                            # padded feature-row length (256 B fp16)

CHUNK = 32768                         # int16 index range per gather chunk
NCHUNK = -(-NODES_PAD // CHUNK)       # 4

GATHER_TILES = int(os.environ.get("GCN_GT", "32"))   # tiles per dma_gather
MSG_BUFS = int(os.environ.get("GCN_MSGBUFS", "3"))

f32 = mybir.dt.float32
f16 = mybir.dt.float16
i32 = mybir.dt.int32
i16 = mybir.dt.int16
AF = mybir.ActivationFunctionType
ALU = mybir.AluOpType

PREP_PASSES = 7                       # xs prep processed 14 slots at a time
PREP_SLOTS = NB // PREP_PASSES        # 14


def _cells_to_grid(cnt_mc, nslots):
    """Shared row grid for one chunk: cell offsets R[s], tile count T.

    cnt_mc: [nslots] max-over-cores counts.  Returns (R, total_rows, T).
    """
    R = np.zeros(nslots, np.int64)
    r = 0
    for s in range(nslots):
        R[s] = r
        r += int(cnt_mc[s])
    return R, r, -(-r // P)


# ---------------------------------------------------------------- host layout
def _build_layout(u, i, edges):
    """Pure index/layout computation (the sharding step)."""
    src = np.asarray(edges[0], dtype=np.int64)
    dst = np.asarray(edges[1], dtype=np.int64)
    u = np.asarray(u, dtype=np.int64)
    i_arr = np.asarray(i, dtype=np.int64)

    # ---- compact item set: the only layer-2 outputs actually read ----
    iset = np.unique(i_arr)                      # sorted
    Ni = len(iset)
    NCB = -(-Ni // P)
    compact = np.full(N_ITEMS, -1, np.int64)
    compact[iset] = np.arange(Ni)

    # ---- layer-2 edges: dst in iset, plus self-loops on iset ----
    m2 = compact[dst] >= 0
    src2 = np.concatenate([src[m2], iset])
    cdst2 = np.concatenate([compact[dst[m2]], np.arange(Ni)])

    # ---- nodes whose z2 is needed: sources of layer-2 edges ----
    needed = np.zeros(N_ITEMS, bool)
    needed[src2] = True

    # ---- layer-1 edges: real edges into needed nodes (no self-loops) ----
    m1 = needed[dst]
    src1, dst1 = src[m1], dst[m1]

    # ---- true degree over ALL original edges ----
    deg_raw = np.bincount(dst, minlength=NODES_PAD).astype(np.int32)

    # ---- block -> (core, slot): greedy by L1 edge count, slot = rank ----
    n_blocks = NODES_PAD // P
    blk_cnt = np.bincount(dst1 // P, minlength=n_blocks)
    order = np.argsort(-blk_cnt, kind="stable")
    core_load = np.zeros(N_CORES, np.int64)
    core_blocks = [[] for _ in range(N_CORES)]
    for b in order:
        cand = [c for c in range(N_CORES) if len(core_blocks[c]) < NB]
        k = min(cand, key=lambda c: core_load[c])
        core_blocks[k].append(b)                  # decreasing size => rank
        core_load[k] += blk_cnt[b]
    assign = np.empty((n_blocks, 2), dtype=np.int64)
    for k in range(N_CORES):
        for s, b in enumerate(core_blocks[k]):
            assign[b] = (k, s)
    core_of_blk, slot_of_blk = assign[:, 0], assign[:, 1]

    nodes = np.arange(NODES_PAD, dtype=np.int64)
    pos_of_node = (core_of_blk[nodes // P] * NODES_PER_CORE
                   + slot_of_blk[nodes // P] * P + nodes % P)
    node_of_pos = np.empty(NODES_PAD, dtype=np.int64)
    node_of_pos[pos_of_node] = nodes

    # ================= layer-1 cells on the shared grid =================
    ecore = core_of_blk[dst1 // P]
    eslot = slot_of_blk[dst1 // P]
    spos1 = pos_of_node[src1]
    echunk = spos1 // CHUNK
    seg = (ecore * NCHUNK + echunk) * NB + eslot
    eo = np.argsort(seg * (NODES_PAD + 1) + spos1, kind="stable")
    seg_s = seg[eo]
    spos_s = spos1[eo]
    dloc_s = (dst1[eo] % P).astype(np.float16)
    nseg = N_CORES * NCHUNK * NB
    s_start = np.searchsorted(seg_s, np.arange(nseg))
    s_end = np.searchsorted(seg_s, np.arange(nseg) + 1)
    cnt1 = (s_end - s_start).reshape(N_CORES, NCHUNK, NB)
    mc1 = cnt1.max(axis=0)                        # [NCHUNK, NB]

    R1 = np.zeros((NCHUNK, NB), np.int64)
    T1 = np.zeros(NCHUNK, np.int64)
    for c in range(NCHUNK):
        R1[c], _, T1[c] = _cells_to_grid(mc1[c], NB)
    CB = np.zeros(NCHUNK + 1, np.int64)
    for c in range(NCHUNK):
        CB[c + 1] = CB[c] + T1[c]
    TOT1 = int(CB[NCHUNK])

    # pieces: (chunk, slot, first_global_tile, col0, npieces)
    plist1 = []
    npc1 = 0
    for c in range(NCHUNK):
        for s in range(NB):
            n = int(mc1[c, s])
            if n == 0:
                continue
            r0, r1 = int(R1[c, s]), int(R1[c, s]) + n
            t0, t1 = r0 // P, (r1 - 1) // P
            plist1.append((c, s, int(CB[c] + t0), npc1, t1 - t0 + 1))
            npc1 += t1 - t0 + 1

    gidx1 = np.zeros((N_CORES, 16, TOT1 * 8), np.int16)
    dstloc1 = np.full((N_CORES, P, npc1), -1.0, np.float16)
    colmap1 = {}                                  # (c, s) -> (t0, col0)
    for (c, s, tg0, col0, npc) in plist1:
        colmap1[(c, s)] = (tg0 - CB[c], col0)
    for k in range(N_CORES):
        for c in range(NCHUNK):
            for s in range(NB):
                sidx = (k * NCHUNK + c) * NB + s
                a, b = int(s_start[sidx]), int(s_end[sidx])
                n = b - a
                if n == 0:
                    continue
                rl = R1[c, s] + np.arange(n)      # chunk-local rows
                jg = CB[c] * P + rl               # global rows
                gidx1[k, jg % 16, jg // 16] = (spos_s[a:b]
                                               - c * CHUNK).astype(np.int16)
                t0, col0 = colmap1[(c, s)]
                dstloc1[k, rl % P, col0 + (rl // P - t0)] = dloc_s[a:b]
    gidx1 = np.tile(gidx1, (1, 8, 1))             # replicate to 128 parts

    # ================= layer-2 cells (src-partitioned, compact dst) =====
    spos2 = pos_of_node[src2]
    ecore2 = spos2 // NODES_PER_CORE
    lrow2 = spos2 % NODES_PER_CORE                # local z2_slice row
    cslot2 = cdst2 // P
    seg2 = ecore2 * NCB + cslot2
    eo2 = np.argsort(seg2 * (NODES_PER_CORE + 1) + lrow2, kind="stable")
    seg2_s = seg2[eo2]
    lrow2_s = lrow2[eo2]
    dloc2_s = (cdst2[eo2] % P).astype(np.float16)
    nseg2 = N_CORES * NCB
    s2_start = np.searchsorted(seg2_s, np.arange(nseg2))
    s2_end = np.searchsorted(seg2_s, np.arange(nseg2) + 1)
    cnt2 = (s2_end - s2_start).reshape(N_CORES, NCB)
    mc2 = cnt2.max(axis=0)                        # [NCB]

    R2, _, T2 = _cells_to_grid(mc2, NCB)
    TOT2 = int(T2)

    plist2 = []
    npc2 = 0
    for s in range(NCB):
        n = int(mc2[s])
        if n == 0:
            continue
        r0, r1 = int(R2[s]), int(R2[s]) + n
        t0, t1 = r0 // P, (r1 - 1) // P
        plist2.append((s, t0, npc2, t1 - t0 + 1))
        npc2 += t1 - t0 + 1

    gidx2 = np.zeros((N_CORES, 16, TOT2 * 8), np.int16)
    dstloc2 = np.full((N_CORES, P, npc2), -1.0, np.float16)
    colmap2 = {s: (t0, col0) for (s, t0, col0, _) in plist2}
    for k in range(N_CORES):
        for s in range(NCB):
            sidx = k * NCB + s
            a, b = int(s2_start[sidx]), int(s2_end[sidx])
            n = b - a
            if n == 0:
                continue
            rl = R2[s] + np.arange(n)
            gidx2[k, rl % 16, rl // 16] = lrow2_s[a:b].astype(np.int16)
            t0, col0 = colmap2[s]
            dstloc2[k, rl % P, col0 + (rl // P - t0)] = dloc2_s[a:b]
    gidx2 = np.tile(gidx2, (1, 8, 1))

    # ================= final-stage buckets by user-owner core ===========
    owner = u // USERS_PER_CORE
    border = np.argsort(owner, kind="stable")
    bstart = np.searchsorted(owner[border], np.arange(N_CORES))
    bend = np.searchsorted(owner[border], np.arange(N_CORES) + 1)
    bmax = int((bend - bstart).max())
    BT = max(1, -(-bmax // P))
    u_loc = np.zeros((N_CORES, P, BT), np.int32)
    i_pos = np.zeros((N_CORES, P, BT), np.int32)
    degi = np.zeros((N_CORES, P, BT), np.int32)
    perm = []
    for k in range(N_CORES):
        sel = border[bstart[k]:bend[k]]
        perm.append(sel)
        jj = np.arange(len(sel))
        u_loc[k, jj % P, jj // P] = u[sel] - k * USERS_PER_CORE
        i_pos[k, jj % P, jj // P] = compact[i_arr[sel]]
        degi[k, jj % P, jj // P] = deg_raw[i_arr[sel]]

    deg_cs = np.zeros((N_CORES, P, NB), dtype=np.int32)
    perm_nodes = node_of_pos.reshape(N_CORES, NB, P)
    for k in range(N_CORES):
        deg_cs[k] = deg_raw[perm_nodes[k]].T

    struct = dict(T1=tuple(int(t) for t in T1), TOT1=TOT1, NPC1=npc1,
                  plist1=tuple(plist1), TOT2=TOT2, NPC2=npc2,
                  plist2=tuple(plist2), NCB=NCB, BT=BT)
    return dict(struct=struct, gidx1=gidx1, dstloc1=dstloc1,
                gidx2=gidx2, dstloc2=dstloc2, deg_cs=deg_cs,
                u_loc=u_loc, i_pos=i_pos, degi=degi, perm=perm,
                node_of_pos=node_of_pos)


# ---------------------------------------------------------------- bass build
def _build_bass(S):
    T1, TOT1, NPC1 = S["T1"], S["TOT1"], S["NPC1"]
    plist1 = S["plist1"]
    TOT2, NPC2, plist2 = S["TOT2"], S["NPC2"], S["plist2"]
    NCB, BT = S["NCB"], S["BT"]
    CB = [0]
    for c in range(NCHUNK):
        CB.append(CB[-1] + T1[c])

    nc = bacc.Bacc("TRN2", target_bir_lowering=False, debug=False,
                   num_devices=N_CORES)

    item_part = nc.dram_tensor("item_part", [NODES_PER_CORE, DIM], f32,
                               kind="ExternalInput")
    users_part = nc.dram_tensor("users_part", [USERS_PER_CORE, DIM], f32,
                                kind="ExternalInput")
    deg_in = nc.dram_tensor("deg", [P, NB], i32, kind="ExternalInput")
    gidx1_in = nc.dram_tensor("gidx1", [P, TOT1 * 8], i16,
                              kind="ExternalInput")
    dstloc1_in = nc.dram_tensor("dstloc1", [P, NPC1], f16,
                                kind="ExternalInput")
    gidx2_in = nc.dram_tensor("gidx2", [P, TOT2 * 8], i16,
                              kind="ExternalInput")
    dstloc2_in = nc.dram_tensor("dstloc2", [P, NPC2], f16,
                                kind="ExternalInput")
    w1_in = nc.dram_tensor("w1", [DIM, HID], f16, kind="ExternalInput")
    w2_in = nc.dram_tensor("w2", [HID, DIM], f16, kind="ExternalInput")
    b1_in = nc.dram_tensor("b1", [P, 1], f32, kind="ExternalInput")
    b2b_in = nc.dram_tensor("b2b", [P, DIM], f32, kind="ExternalInput")
    iota_in = nc.dram_tensor("iotaf", [P, P], f16, kind="ExternalInput")
    ident_in = nc.dram_tensor("identf", [P, P], f16, kind="ExternalInput")
    uloc_in = nc.dram_tensor("u_loc", [P, BT], i32, kind="ExternalInput")
    ipos_in = nc.dram_tensor("i_pos", [P, BT], i32, kind="ExternalInput")
    degi_in = nc.dram_tensor("degi", [P, BT], i32, kind="ExternalInput")
    out_t = nc.dram_tensor("out", [P, BT], f32, kind="ExternalOutput")

    rg = [list(range(N_CORES))]

    groups2 = []
    t = 0
    while t < TOT2:
        ntl = min(GATHER_TILES, TOT2 - t)
        groups2.append((t, ntl))
        t += ntl
    gmax = GATHER_TILES

    def group_of(tg, groups, key):
        for g in groups:
            if key(g)[0] <= tg < key(g)[0] + key(g)[1]:
                return key(g)[0]
        raise AssertionError(tg)

    with tile.TileContext(nc) as tc:
        with (
            tc.tile_pool(name="const", bufs=1) as cpool,
            tc.tile_pool(name="prep", bufs=2) as ppool,
            tc.tile_pool(name="acc", bufs=1) as accpool,
            tc.tile_pool(name="msg", bufs=MSG_BUFS) as msgpool,
            tc.tile_pool(name="oneh", bufs=4) as spool,
            tc.tile_pool(name="flush", bufs=3) as fpool,
            tc.tile_pool(name="pag", bufs=3, space="PSUM") as pagg,
            tc.tile_pool(name="pfa", bufs=2, space="PSUM") as pfa,
            tc.tile_pool(name="pfb", bufs=2, space="PSUM") as pfb,
            tc.tile_pool(name="dram", bufs=1, space="DRAM") as dpool,
        ):
            # ---------- constants ----------
            iota_sb = cpool.tile([P, P], f16)
            nc.sync.dma_start(iota_sb[:], iota_in[:, :])
            ident_sb = cpool.tile([P, P], f16)
            nc.sync.dma_start(ident_sb[:], ident_in[:, :])
            w1_sb = cpool.tile([DIM, HID], f16)
            nc.sync.dma_start(w1_sb[:], w1_in[:, :])
            w2_sb = cpool.tile([HID, DIM], f16)
            nc.sync.dma_start(w2_sb[:], w2_in[:, :])
            b1_sb = cpool.tile([P, 1], f32)
            nc.sync.dma_start(b1_sb[:], b1_in[:, :])
            b2b_sb = cpool.tile([P, DIM], f32)
            nc.sync.dma_start(b2b_sb[:], b2b_in[:, :])

            gidx1_sb = cpool.tile([P, TOT1 * 8], i16)
            nc.sync.dma_start(gidx1_sb[:], gidx1_in[:, :])
            dstloc1_sb = cpool.tile([P, NPC1], f16)
            nc.sync.dma_start(dstloc1_sb[:], dstloc1_in[:, :])
            gidx2_sb = cpool.tile([P, TOT2 * 8], i16)
            nc.sync.dma_start(gidx2_sb[:], gidx2_in[:, :])
            dstloc2_sb = cpool.tile([P, NPC2], f16)
            nc.sync.dma_start(dstloc2_sb[:], dstloc2_in[:, :])

            uloc_sb = cpool.tile([P, BT], i32)
            nc.sync.dma_start(uloc_sb[:], uloc_in[:, :])
            ipos_sb = cpool.tile([P, BT], i32)
            nc.sync.dma_start(ipos_sb[:], ipos_in[:, :])
            degi_sb = cpool.tile([P, BT], i32)
            nc.sync.dma_start(degi_sb[:], degi_in[:, :])

            # ---------- dinv ----------
            deg_sb = cpool.tile([P, NB], i32)
            nc.sync.dma_start(deg_sb[:], deg_in[:, :])
            degf = cpool.tile([P, NB], f32)
            nc.vector.tensor_copy(degf[:], deg_sb[:])
            dsq = cpool.tile([P, NB], f32)
            nc.scalar.activation(dsq[:], degf[:], AF.Sqrt, bias=1.0, scale=1.0)
            dinv_sb = cpool.tile([P, NB], f32)
            nc.vector.reciprocal(dinv_sb[:], dsq[:])

            # ---------- final-phase user gathers (early: Q7 idle now) ----
            us = cpool.tile([P, BT * DIM], f32, tag="us")
            for t in range(BT):
                nc.gpsimd.indirect_dma_start(
                    out=us[:, t * DIM:(t + 1) * DIM],
                    out_offset=None, in_=users_part[:, :],
                    in_offset=bass.IndirectOffsetOnAxis(
                        ap=uloc_sb[:, t:t + 1], axis=0))

            # ---------- xs slice ----------
            acc = accpool.tile([P, NB * DIM], f16, tag="acc")
            for pp in range(PREP_PASSES):
                s0 = pp * PREP_SLOTS
                ns = PREP_SLOTS
                item_sb = ppool.tile([P, ns * DIM], f32, tag="item")
                nc.sync.dma_start(
                    item_sb[:].rearrange("p (j f) -> p j f", f=DIM),
                    item_part[:, :].rearrange("(j p) f -> p j f", p=P)
                    [:, s0:s0 + ns, :])
                sq = ppool.tile([P, ns * DIM], f32, tag="sq")
                nc.vector.tensor_tensor(sq[:], item_sb[:], item_sb[:],
                                        ALU.mult)
                ss = ppool.tile([P, ns], f32, tag="ss")
                nc.vector.tensor_reduce(
                    ss[:], sq[:].rearrange("p (j f) -> p j f", f=DIM),
                    axis=mybir.AxisListType.X, op=ALU.add)
                nrm = ppool.tile([P, ns], f32, tag="nrm")
                nc.scalar.activation(nrm[:], ss[:], AF.Sqrt)
                nrm1 = ppool.tile([P, ns], f32, tag="nrm1")
                nc.vector.tensor_scalar_max(nrm1[:], nrm[:], 1.0)
                rinv = ppool.tile([P, ns], f32, tag="rinv")
                nc.vector.reciprocal(rinv[:], nrm1[:])
                scal = ppool.tile([P, ns], f32, tag="scal")
                nc.vector.tensor_tensor(scal[:], rinv[:],
                                        dinv_sb[:, s0:s0 + ns], ALU.mult)
                # 2D first-column op absorbs waits; 3D op needs <=1 wait
                nc.vector.tensor_tensor(acc[:, s0 * DIM:s0 * DIM + 1],
                                        item_sb[:, 0:1], scal[:, 0:1],
                                        ALU.mult)
                nc.vector.tensor_tensor(
                    acc[:, s0 * DIM:(s0 + ns) * DIM].rearrange(
                        "p (j f) -> p j f", f=DIM),
                    item_sb[:].rearrange("p (j f) -> p j f", f=DIM),
                    scal[:, :, None].to_broadcast([P, ns, DIM]),
                    ALU.mult)

            xs_slice = dpool.tile([NODES_PER_CORE, ROWP], f16)
            nc.sync.dma_start(
                xs_slice[:].rearrange("(j p) F -> p j F", p=P)[:, :, 0:DIM],
                acc[:].rearrange("p (j f) -> p j f", f=DIM))
            xs_full = dpool.tile([NODES_PAD, ROWP], f16, addr_space="Shared")
            nc.gpsimd.collective_compute(
                "AllGather", ALU.bypass, replica_groups=rg,
                ins=[xs_slice.opt()], outs=[xs_full.opt()])

            # self-loop term: aggall starts at xs (f16 -> f32 copy)
            aggall = accpool.tile([P, NB * DIM], f32, tag="aggall")
            nc.vector.tensor_copy(aggall[:], acc[:])

            # final-phase user-side math (overlaps the layer-1 grind)
            ub2 = cpool.tile([P, BT * DIM], f32, tag="ub2")
            nc.vector.tensor_tensor(
                ub2[:].rearrange("p (t f) -> p t f", f=DIM),
                us[:].rearrange("p (t f) -> p t f", f=DIM),
                b2b_sb[:, None, :].to_broadcast([P, BT, DIM]), ALU.mult)
            db2 = cpool.tile([P, BT], f32)
            nc.vector.tensor_reduce(
                db2[:], ub2[:].rearrange("p (t f) -> p t f", f=DIM),
                axis=mybir.AxisListType.X, op=ALU.add)
            usq = cpool.tile([P, BT * DIM], f32, tag="usq")
            nc.vector.tensor_tensor(usq[:], us[:], us[:], ALU.mult)
            ssu = cpool.tile([P, BT], f32)
            nc.vector.tensor_reduce(
                ssu[:], usq[:].rearrange("p (t f) -> p t f", f=DIM),
                axis=mybir.AxisListType.X, op=ALU.add)
            un = cpool.tile([P, BT], f32)
            nc.scalar.activation(un[:], ssu[:], AF.Sqrt)
            un1 = cpool.tile([P, BT], f32)
            nc.vector.tensor_scalar_max(un1[:], un[:], 1.0)
            ur = cpool.tile([P, BT], f32)
            nc.vector.reciprocal(ur[:], un1[:])
            degif = cpool.tile([P, BT], f32)
            nc.vector.tensor_copy(degif[:], degi_sb[:])
            dgs = cpool.tile([P, BT], f32)
            nc.scalar.activation(dgs[:], degif[:], AF.Sqrt, bias=1.0,
                                 scale=1.0)
            dinvi = cpool.tile([P, BT], f32)
            nc.vector.reciprocal(dinvi[:], dgs[:])

            # ---------- layer 1: window-interleaved gather/agg/flush ----
            cells1 = {}
            for (c, s, tg0, col0, npc) in plist1:
                cells1[(c, s)] = (tg0, col0, npc)

            def flush_slot(s):
                sl = slice(s * DIM, (s + 1) * DIM)
                aggsb = fpool.tile([P, DIM], f16, tag="aggsb")
                nc.scalar.activation(aggsb[:], aggall[:, sl], AF.Copy,
                                     scale=dinv_sb[:, s:s + 1])
                aggT = pfa.tile([DIM, P], f16, tag="pfa")
                nc.tensor.transpose(aggT[:], aggsb[:], ident_sb[:])
                aggT_sb = fpool.tile([DIM, P], f16, tag="aggT")
                nc.scalar.activation(aggT_sb[:], aggT[:], AF.Copy)
                U = pfb.tile([P, P], f32, tag="pfb")
                nc.tensor.matmul(U[:], lhsT=w1_sb[:], rhs=aggT_sb[:],
                                 start=True, stop=True)
                H = fpool.tile([P, P], f16, tag="H")
                nc.scalar.activation(H[:], U[:], AF.Relu,
                                     bias=b1_sb[:, 0:1], scale=1.0)
                z2p = pfa.tile([DIM, P], f32, tag="pfa")
                nc.tensor.matmul(z2p[:], lhsT=w2_sb[:], rhs=H[:],
                                 start=True, stop=True)
                z2p_sb = fpool.tile([DIM, P], f16, tag="z2p")
                nc.scalar.activation(z2p_sb[:], z2p[:], AF.Copy)
                z2r = pfb.tile([P, DIM], f16, tag="pfb")
                nc.tensor.transpose(z2r[:], z2p_sb[:],
                                    ident_sb[:DIM, :DIM])
                nc.scalar.activation(acc[:, sl], z2r[:], AF.Copy,
                                     scale=dinv_sb[:, s:s + 1])

            z2_slice = dpool.tile([NODES_PER_CORE, ROWP], f16)
            msgs1 = {}
            tile2grp = {}
            WS = PREP_SLOTS
            for w0 in range(0, NB, WS):
                ws = range(w0, min(w0 + WS, NB))
                # issue gathers chunk-major (boundary tiles re-gathered)
                for c in range(NCHUNK):
                    lo, hi = None, -1
                    for s in ws:
                        if (c, s) in cells1:
                            tg0, _, npc = cells1[(c, s)]
                            lo = tg0 if lo is None else min(lo, tg0)
                            hi = max(hi, tg0 + npc - 1)
                    if lo is None:
                        continue
                    t = lo
                    crows = min(CHUNK, NODES_PAD - c * CHUNK)
                    while t <= hi:
                        ntl = min(GATHER_TILES, hi + 1 - t)
                        m = msgpool.tile([P, gmax * ROWP], f16, tag="msg")
                        nc.gpsimd.dma_gather(
                            m[:, :ntl * ROWP].rearrange(
                                "p (t F) -> p t F", F=ROWP),
                            xs_full[c * CHUNK:c * CHUNK + crows, :],
                            gidx1_sb[:, t * 8:(t + ntl) * 8],
                            ntl * P, ntl * P, ROWP, single_packet=False)
                        msgs1[t] = m
                        for tt in range(t, t + ntl):
                            tile2grp[tt] = t
                        t += ntl
                # aggregate chunk-major (must match gather issue order)
                for c in range(NCHUNK):
                    for s in ws:
                        if (c, s) not in cells1:
                            continue
                        tg0, col0, npc = cells1[(c, s)]
                        agg = pagg.tile([P, DIM], f32, tag="agg")
                        for pi in range(npc):
                            tg = tg0 + pi
                            g0 = tile2grp[tg]
                            m = msgs1[g0]
                            tl = tg - g0
                            Sx = spool.tile([P, P], f16, tag="oneh")
                            nc.vector.tensor_tensor(
                                Sx[:],
                                dstloc1_sb[:, col0 + pi:col0 + pi + 1]
                                .to_broadcast([P, P]),
                                iota_sb[:], ALU.is_equal)
                            nc.tensor.matmul(
                                agg[:], lhsT=Sx[:],
                                rhs=m[:].rearrange("p (t F) -> p t F",
                                                   F=ROWP)[:, tl, 0:DIM],
                                start=(pi == 0), stop=(pi == npc - 1))
                        sl = slice(s * DIM, (s + 1) * DIM)
                        nc.vector.tensor_tensor(aggall[:, sl], aggall[:, sl],
                                                agg[:], ALU.add)
                # flush + z2 window write (hides under later windows' Q7)
                for s in ws:
                    flush_slot(s)
                nw = len(ws)
                nc.sync.dma_start(
                    z2_slice[:].rearrange("(j p) F -> p j F", p=P)
                    [:, w0:w0 + nw, 0:DIM],
                    acc[:, w0 * DIM:(w0 + nw) * DIM]
                    .rearrange("p (j f) -> p j f", f=DIM))

            # ---------- layer 2: gathers from local z2 ----------
            msgs2 = {}
            for (t0, ntl) in groups2:
                m = msgpool.tile([P, gmax * ROWP], f16, tag="msg")
                nc.gpsimd.dma_gather(
                    m[:, :ntl * ROWP].rearrange("p (t F) -> p t F", F=ROWP),
                    z2_slice[:, :],
                    gidx2_sb[:, t0 * 8:(t0 + ntl) * 8],
                    ntl * P, ntl * P, ROWP, single_packet=False)
                msgs2[t0] = m

            # ---------- layer 2: piece aggregation into compact ----------
            compact_sb = accpool.tile([P, NCB * DIM], f16, tag="compact")
            for (s, t0c, col0, npc) in plist2:
                agg = pagg.tile([P, DIM], f32, tag="agg")
                for pi in range(npc):
                    tg = t0c + pi
                    g0 = group_of(tg, groups2, lambda g: (g[0], g[1]))
                    m = msgs2[g0]
                    tl = tg - g0
                    Sx = spool.tile([P, P], f16, tag="oneh")
                    nc.vector.tensor_tensor(
                        Sx[:],
                        dstloc2_sb[:, col0 + pi:col0 + pi + 1]
                        .to_broadcast([P, P]),
                        iota_sb[:], ALU.is_equal)
                    nc.tensor.matmul(
                        agg[:], lhsT=Sx[:],
                        rhs=m[:].rearrange("p (t F) -> p t F",
                                           F=ROWP)[:, tl, 0:DIM],
                        start=(pi == 0), stop=(pi == npc - 1))
                nc.scalar.activation(compact_sb[:, s * DIM:(s + 1) * DIM],
                                     agg[:], AF.Copy)

            # ---------- compact partials -> AllReduce (f16) ----------
            part_c = dpool.tile([NCB * P, DIM], f16)
            nc.sync.dma_start(
                part_c[:].rearrange("(j p) f -> p j f", p=P),
                compact_sb[:].rearrange("p (j f) -> p j f", f=DIM))
            ar_c = dpool.tile([NCB * P, DIM], f16, addr_space="Shared")
            nc.gpsimd.collective_compute(
                "AllReduce", ALU.add, replica_groups=rg,
                ins=[part_c.opt()], outs=[ar_c.opt()])

            # ---------- final: item gather + dot + sigmoid ----------
            it = cpool.tile([P, BT * DIM], f16, tag="it")
            for t in range(BT):
                nc.gpsimd.indirect_dma_start(
                    out=it[:, t * DIM:(t + 1) * DIM],
                    out_offset=None, in_=ar_c[:],
                    in_offset=bass.IndirectOffsetOnAxis(
                        ap=ipos_sb[:, t:t + 1], axis=0))
            it32 = cpool.tile([P, BT * DIM], f32, tag="it32")
            nc.vector.tensor_copy(it32[:], it[:])

            pr = cpool.tile([P, BT * DIM], f32, tag="pr")
            nc.vector.tensor_tensor(pr[:], us[:], it32[:], ALU.mult)
            d = cpool.tile([P, BT], f32)
            nc.vector.tensor_reduce(
                d[:], pr[:].rearrange("p (t f) -> p t f", f=DIM),
                axis=mybir.AxisListType.X, op=ALU.add)

            dsc = cpool.tile([P, BT], f32)
            nc.vector.tensor_tensor(dsc[:], d[:], dinvi[:], ALU.mult)
            dtot = cpool.tile([P, BT], f32)
            nc.vector.tensor_tensor(dtot[:], dsc[:], db2[:], ALU.add)

            d2 = cpool.tile([P, BT], f32)
            nc.vector.tensor_tensor(d2[:], dtot[:], ur[:], ALU.mult)
            sg = cpool.tile([P, BT], f32)
            nc.scalar.activation(sg[:], d2[:], AF.Sigmoid)
            nc.sync.dma_start(out_t[:, :], sg[:])

    nc.compile()
    return nc


def _legalize_waits(nc):
    """Split multi-sem waits off compute instructions onto preceding NoOps.

    walrus codegen's per-instruction ISA structs have a single sync-wait
    slot; Tile can attach several.  Standalone NoOps carry the extras (one
    wait per NoOp, executed in engine program order just before the
    instruction, so semantics are unchanged).
    """
    keep = (mybir.InstCollectiveCompute, mybir.InstNoOp,
            mybir.InstEventSemaphore, mybir.InstAllEngineBarrier)
    fn = nc.m.functions[0]
    n_split = 0
    for blk in fn.blocks:
        insts = list(blk.instructions)
        out = []
        changed = False
        for inst in insts:
            si = inst.sync_info
            if (si is not None and len(si.on_wait) > 1
                    and not isinstance(inst, keep)):
                waits = list(si.on_wait)
                for w_i, w in enumerate(waits[:-1]):
                    nop = mybir.InstNoOp(
                        name=f"wnop{w_i}_{inst.name}", ins=[], outs=[])
                    nop.engine = inst.engine
                    nop.sync_info = mybir.SyncInfo(on_wait=[w], on_update=[])
                    out.append(nop)
                inst.sync_info = mybir.SyncInfo(
                    on_wait=[waits[-1]], on_update=list(si.on_update))
                changed = True
                n_split += 1
            out.append(inst)
        if changed:
            blk.instructions = out
    return n_split


# ---------------------------------------------------------------- runner
_BUILD_CACHE = {}


def _get_bass(struct):
    key = (struct["T1"], struct["TOT1"], struct["NPC1"], struct["plist1"],
           struct["TOT2"], struct["NPC2"], struct["plist2"],
           struct["NCB"], struct["BT"])
    if key not in _BUILD_CACHE:
        nc = _build_bass(struct)
        _legalize_waits(nc)
        _BUILD_CACHE[key] = nc
    return _BUILD_CACHE[key]


def _make_in_maps(L, item_emb, users_emb, W1, b1, W2, b2):
    item_pad = np.zeros((NODES_PAD, DIM), np.float32)
    item_pad[:N_ITEMS] = np.asarray(item_emb, np.float32)
    item_perm = item_pad[L["node_of_pos"]]

    w1h = np.asarray(W1, np.float32).astype(np.float16)
    w2h = np.asarray(W2, np.float32).astype(np.float16)
    b1c = np.asarray(b1, np.float32).reshape(HID, 1)
    b2b = np.broadcast_to(np.asarray(b2, np.float32), (P, DIM)).copy()
    iotaf = np.broadcast_to(np.arange(P, dtype=np.float16), (P, P)).copy()
    identf = np.eye(P, dtype=np.float16)
    users = np.asarray(users_emb, np.float32)

    in_maps = []
    for k in range(N_CORES):
        in_maps.append({
            "item_part": item_perm[k * NODES_PER_CORE:(k + 1) * NODES_PER_CORE],
            "users_part": users[k * USERS_PER_CORE:(k + 1) * USERS_PER_CORE],
            "deg": L["deg_cs"][k],
            "gidx1": L["gidx1"][k], "dstloc1": L["dstloc1"][k],
            "gidx2": L["gidx2"][k], "dstloc2": L["dstloc2"][k],
            "w1": w1h, "w2": w2h, "b1": b1c, "b2b": b2b,
            "iotaf": iotaf, "identf": identf,
            "u_loc": L["u_loc"][k], "i_pos": L["i_pos"][k],
            "degi": L["degi"][k],
        })
    return in_maps


def _ensure_ntff_hook():
    """Register the axon NTFF profiling hook if the image lacks
    antenv.axon_hooks (needed for trace=True under axon)."""
    import sys
    import types
    try:
        from antenv.axon_hooks import get_axon_ntff_profile_hook  # noqa: F401
        return
    except ImportError:
        pass
    try:
        import antenv
        from trn_agent_boot.trn_boot import _ntff_profile_via_ctypes
        hook = _ntff_profile_via_ctypes("/opt/axon/libaxon_pjrt.so")
        mod = types.ModuleType("antenv.axon_hooks")
        box = {"v": hook}
        mod.set_axon_ntff_profile_hook = lambda h: box.__setitem__("v", h)
        mod.get_axon_ntff_profile_hook = lambda: box["v"]
        sys.modules["antenv.axon_hooks"] = mod
        antenv.axon_hooks = mod
    except Exception:
        pass


def run(u, i, edges, users_emb, item_emb, W1, b1, W2, b2, trace=False):
    if trace:
        _ensure_ntff_hook()
    L = _build_layout(u, i, edges)
    nc = _get_bass(L["struct"])
    in_maps = _make_in_maps(L, item_emb, users_emb, W1, b1, W2, b2)
    res = run_bass_kernel_spmd(nc, in_maps, core_ids=list(range(N_CORES)),
                               trace=trace)
    logits = np.zeros(B_PAIRS, np.float32)
    for k in range(N_CORES):
        sel = L["perm"][k]
        o = np.asarray(res.results[k]["out"])
        logits[sel] = o.T.ravel()[:len(sel)]
    return logits, res


def kernel(u, i, edges, users_emb, item_emb, W1, b1, W2, b2):
    logits, _ = run(u, i, edges, users_emb, item_emb, W1, b1, W2, b2,
                    trace=bool(int(os.environ.get("GCN_TRACE", "0"))))
    return logits


# revision 20
# speedup vs baseline: 2.5007x; 1.0113x over previous
"""Trainium2 Bass kernel for nn_GCN4Rec (2-layer GCNConv + user/item dot).

Sharding (8 NeuronCores):
  - item/node space padded to 100352 rows; each core owns 98 blocks of 128
    nodes (blocks assigned to cores by a greedy layer-1 edge balancer; slot
    index = per-core size rank so per-(chunk,slot) counts align across cores).
  - Layer 1 is dst-partitioned: graph edges whose dst produces a needed z2
    row (sources of layer-2 edges) are bucketed per (core, src-chunk, slot),
    sorted by source row, and laid out on a shared row grid (max count over
    cores per cell, 1-row granularity).  Tiles of 128 rows may span slot
    boundaries; each (tile, slot) intersection is a "piece" with its own
    masked one-hot column (-1 rows contribute nothing).
  - Self-loops are never gathered: the aggregation accumulator is initialized
    from the local xs block already in SBUF.
  - Layer 2 is restricted to dst in unique(i) (the only rows the output
    reads) and src-partitioned: each core gathers from its LOCAL z2 slice
    (single int16 index chunk, no AllGather) and aggregates partial sums into
    a compact [~Ni, 64] table; partials are summed with one AllReduce.  The
    dst-side deg^{-1/2} and b2 are folded into the final per-pair dot.
  - GCN math is reformulated so aggregation is a pure segment-sum of
    pre-scaled features (xs = max_norm(item_emb) * dinv); the Linear layers
    are applied per destination block after aggregation.  The segment-sum
    runs on the tensor engine as one-hot matmuls accumulating in PSUM;
    per-edge rows are fetched with bulk dma_gather (Q7 SWDGE ucode).
  - users_emb is row-sharded (125k rows/core); the B=16384 pairs are bucketed
    by owning core of u; item vectors are gathered from the AllReduced
    compact table.

kernel(**inputs) takes the full unsharded inputs and returns the full [B]
sigmoid logits, matching reference.py.
"""
import os
import numpy as np

import concourse.bass as bass
import concourse.bacc as bacc
import concourse.mybir as mybir
import concourse.tile as tile
from concourse.bass_utils import run_bass_kernel_spmd

# ---------------------------------------------------------------- constants
N_USERS = 1_000_000
N_ITEMS = 100_000
DIM = 64
HID = 128
B_PAIRS = 16_384

N_CORES = 8
P = 128
NB = 98                               # blocks (slots) per core
NODES_PAD = N_CORES * NB * P          # 100352
NODES_PER_CORE = NB * P               # 12544
USERS_PER_CORE = N_USERS // N_CORES   # 125000
ROWP = 128                            # padded feature-row length (256 B fp16)

CHUNK = 32768                         # int16 index range per gather chunk
NCHUNK = -(-NODES_PAD // CHUNK)       # 4

GATHER_TILES = int(os.environ.get("GCN_GT", "48"))   # tiles per dma_gather
MSG_BUFS = int(os.environ.get("GCN_MSGBUFS", "3"))
WINDOW_SLOTS = int(os.environ.get("GCN_WS", "16"))   # L1 pipeline window

f32 = mybir.dt.float32
f16 = mybir.dt.float16
i32 = mybir.dt.int32
i16 = mybir.dt.int16
AF = mybir.ActivationFunctionType
ALU = mybir.AluOpType

PREP_PASSES = 7                       # xs prep processed 14 slots at a time
PREP_SLOTS = NB // PREP_PASSES        # 14


def _cells_to_grid(cnt_mc, nslots):
    """Shared row grid for one chunk: cell offsets R[s], tile count T.

    cnt_mc: [nslots] max-over-cores counts.  Returns (R, total_rows, T).
    """
    R = np.zeros(nslots, np.int64)
    r = 0
    for s in range(nslots):
        R[s] = r
        r += int(cnt_mc[s])
    return R, r, -(-r // P)


# ---------------------------------------------------------------- host layout
def _build_layout(u, i, edges):
    """Pure index/layout computation (the sharding step)."""
    src = np.asarray(edges[0], dtype=np.int64)
    dst = np.asarray(edges[1], dtype=np.int64)
    u = np.asarray(u, dtype=np.int64)
    i_arr = np.asarray(i, dtype=np.int64)

    # ---- compact item set: the only layer-2 outputs actually read ----
    iset = np.unique(i_arr)                      # sorted
    Ni = len(iset)
    NCB = -(-Ni // P)
    compact = np.full(N_ITEMS, -1, np.int64)
    compact[iset] = np.arange(Ni)

    # ---- layer-2 edges: dst in iset, plus self-loops on iset ----
    m2 = compact[dst] >= 0
    src2 = np.concatenate([src[m2], iset])
    cdst2 = np.concatenate([compact[dst[m2]], np.arange(Ni)])

    # ---- nodes whose z2 is needed: sources of layer-2 edges ----
    needed = np.zeros(N_ITEMS, bool)
    needed[src2] = True

    # ---- layer-1 edges: real edges into needed nodes (no self-loops) ----
    m1 = needed[dst]
    src1, dst1 = src[m1], dst[m1]

    # ---- true degree over ALL original edges ----
    deg_raw = np.bincount(dst, minlength=NODES_PAD).astype(np.int32)

    # ---- block -> (core, slot): greedy by L1 edge count, slot = rank ----
    n_blocks = NODES_PAD // P
    blk_cnt = np.bincount(dst1 // P, minlength=n_blocks)
    order = np.argsort(-blk_cnt, kind="stable")
    core_load = np.zeros(N_CORES, np.int64)
    core_blocks = [[] for _ in range(N_CORES)]
    for b in order:
        cand = [c for c in range(N_CORES) if len(core_blocks[c]) < NB]
        k = min(cand, key=lambda c: core_load[c])
        core_blocks[k].append(b)                  # decreasing size => rank
        core_load[k] += blk_cnt[b]
    assign = np.empty((n_blocks, 2), dtype=np.int64)
    for k in range(N_CORES):
        for s, b in enumerate(core_blocks[k]):
            assign[b] = (k, s)
    core_of_blk, slot_of_blk = assign[:, 0], assign[:, 1]

    nodes = np.arange(NODES_PAD, dtype=np.int64)
    pos_of_node = (core_of_blk[nodes // P] * NODES_PER_CORE
                   + slot_of_blk[nodes // P] * P + nodes % P)
    node_of_pos = np.empty(NODES_PAD, dtype=np.int64)
    node_of_pos[pos_of_node] = nodes

    # ================= layer-1 cells on the shared grid =================
    ecore = core_of_blk[dst1 // P]
    eslot = slot_of_blk[dst1 // P]
    spos1 = pos_of_node[src1]
    echunk = spos1 // CHUNK
    seg = (ecore * NCHUNK + echunk) * NB + eslot
    eo = np.argsort(seg * (NODES_PAD + 1) + spos1, kind="stable")
    seg_s = seg[eo]
    spos_s = spos1[eo]
    dloc_s = (dst1[eo] % P).astype(np.float16)
    nseg = N_CORES * NCHUNK * NB
    s_start = np.searchsorted(seg_s, np.arange(nseg))
    s_end = np.searchsorted(seg_s, np.arange(nseg) + 1)
    cnt1 = (s_end - s_start).reshape(N_CORES, NCHUNK, NB)
    mc1 = cnt1.max(axis=0)                        # [NCHUNK, NB]

    R1 = np.zeros((NCHUNK, NB), np.int64)
    T1 = np.zeros(NCHUNK, np.int64)
    for c in range(NCHUNK):
        R1[c], _, T1[c] = _cells_to_grid(mc1[c], NB)
    CB = np.zeros(NCHUNK + 1, np.int64)
    for c in range(NCHUNK):
        CB[c + 1] = CB[c] + T1[c]
    TOT1 = int(CB[NCHUNK])

    # pieces: (chunk, slot, first_global_tile, col0, npieces)
    plist1 = []
    npc1 = 0
    for c in range(NCHUNK):
        for s in range(NB):
            n = int(mc1[c, s])
            if n == 0:
                continue
            r0, r1 = int(R1[c, s]), int(R1[c, s]) + n
            t0, t1 = r0 // P, (r1 - 1) // P
            plist1.append((c, s, int(CB[c] + t0), npc1, t1 - t0 + 1))
            npc1 += t1 - t0 + 1

    gidx1 = np.zeros((N_CORES, 16, TOT1 * 8), np.int16)
    dstloc1 = np.full((N_CORES, P, npc1), -1.0, np.float16)
    colmap1 = {}                                  # (c, s) -> (t0, col0)
    for (c, s, tg0, col0, npc) in plist1:
        colmap1[(c, s)] = (tg0 - CB[c], col0)
    for k in range(N_CORES):
        for c in range(NCHUNK):
            for s in range(NB):
                sidx = (k * NCHUNK + c) * NB + s
                a, b = int(s_start[sidx]), int(s_end[sidx])
                n = b - a
                if n == 0:
                    continue
                rl = R1[c, s] + np.arange(n)      # chunk-local rows
                jg = CB[c] * P + rl               # global rows
                gidx1[k, jg % 16, jg // 16] = (spos_s[a:b]
                                               - c * CHUNK).astype(np.int16)
                t0, col0 = colmap1[(c, s)]
                dstloc1[k, rl % P, col0 + (rl // P - t0)] = dloc_s[a:b]
    gidx1 = np.tile(gidx1, (1, 8, 1))             # replicate to 128 parts

    # ================= layer-2 cells (src-partitioned, compact dst) =====
    spos2 = pos_of_node[src2]
    ecore2 = spos2 // NODES_PER_CORE
    lrow2 = spos2 % NODES_PER_CORE                # local z2_slice row
    cslot2 = cdst2 // P
    seg2 = ecore2 * NCB + cslot2
    eo2 = np.argsort(seg2 * (NODES_PER_CORE + 1) + lrow2, kind="stable")
    seg2_s = seg2[eo2]
    lrow2_s = lrow2[eo2]
    dloc2_s = (cdst2[eo2] % P).astype(np.float16)
    nseg2 = N_CORES * NCB
    s2_start = np.searchsorted(seg2_s, np.arange(nseg2))
    s2_end = np.searchsorted(seg2_s, np.arange(nseg2) + 1)
    cnt2 = (s2_end - s2_start).reshape(N_CORES, NCB)
    mc2 = cnt2.max(axis=0)                        # [NCB]

    R2, _, T2 = _cells_to_grid(mc2, NCB)
    TOT2 = int(T2)

    plist2 = []
    npc2 = 0
    for s in range(NCB):
        n = int(mc2[s])
        if n == 0:
            continue
        r0, r1 = int(R2[s]), int(R2[s]) + n
        t0, t1 = r0 // P, (r1 - 1) // P
        plist2.append((s, t0, npc2, t1 - t0 + 1))
        npc2 += t1 - t0 + 1

    gidx2 = np.zeros((N_CORES, 16, TOT2 * 8), np.int16)
    dstloc2 = np.full((N_CORES, P, npc2), -1.0, np.float16)
    colmap2 = {s: (t0, col0) for (s, t0, col0, _) in plist2}
    for k in range(N_CORES):
        for s in range(NCB):
            sidx = k * NCB + s
            a, b = int(s2_start[sidx]), int(s2_end[sidx])
            n = b - a
            if n == 0:
                continue
            rl = R2[s] + np.arange(n)
            gidx2[k, rl % 16, rl // 16] = lrow2_s[a:b].astype(np.int16)
            t0, col0 = colmap2[s]
            dstloc2[k, rl % P, col0 + (rl // P - t0)] = dloc2_s[a:b]
    gidx2 = np.tile(gidx2, (1, 8, 1))

    # ================= final-stage buckets by user-owner core ===========
    owner = u // USERS_PER_CORE
    border = np.argsort(owner, kind="stable")
    bstart = np.searchsorted(owner[border], np.arange(N_CORES))
    bend = np.searchsorted(owner[border], np.arange(N_CORES) + 1)
    bmax = int((bend - bstart).max())
    BT = max(1, -(-bmax // P))
    u_loc = np.zeros((N_CORES, P, BT), np.int32)
    i_pos = np.zeros((N_CORES, P, BT), np.int32)
    degi = np.zeros((N_CORES, P, BT), np.int32)
    perm = []
    for k in range(N_CORES):
        sel = border[bstart[k]:bend[k]]
        perm.append(sel)
        jj = np.arange(len(sel))
        u_loc[k, jj % P, jj // P] = u[sel] - k * USERS_PER_CORE
        i_pos[k, jj % P, jj // P] = compact[i_arr[sel]]
        degi[k, jj % P, jj // P] = deg_raw[i_arr[sel]]

    deg_cs = np.zeros((N_CORES, P, NB), dtype=np.int32)
    perm_nodes = node_of_pos.reshape(N_CORES, NB, P)
    for k in range(N_CORES):
        deg_cs[k] = deg_raw[perm_nodes[k]].T

    struct = dict(T1=tuple(int(t) for t in T1), TOT1=TOT1, NPC1=npc1,
                  plist1=tuple(plist1), TOT2=TOT2, NPC2=npc2,
                  plist2=tuple(plist2), NCB=NCB, BT=BT)
    return dict(struct=struct, gidx1=gidx1, dstloc1=dstloc1,
                gidx2=gidx2, dstloc2=dstloc2, deg_cs=deg_cs,
                u_loc=u_loc, i_pos=i_pos, degi=degi, perm=perm,
                node_of_pos=node_of_pos)


# ---------------------------------------------------------------- bass build
def _build_bass(S):
    T1, TOT1, NPC1 = S["T1"], S["TOT1"], S["NPC1"]
    plist1 = S["plist1"]
    TOT2, NPC2, plist2 = S["TOT2"], S["NPC2"], S["plist2"]
    NCB, BT = S["NCB"], S["BT"]
    CB = [0]
    for c in range(NCHUNK):
        CB.append(CB[-1] + T1[c])

    nc = bacc.Bacc("TRN2", target_bir_lowering=False, debug=False,
                   num_devices=N_CORES)

    item_part = nc.dram_tensor("item_part", [NODES_PER_CORE, DIM], f32,
                               kind="ExternalInput")
    users_part = nc.dram_tensor("users_part", [USERS_PER_CORE, DIM], f32,
                                kind="ExternalInput")
    deg_in = nc.dram_tensor("deg", [P, NB], i32, kind="ExternalInput")
    gidx1_in = nc.dram_tensor("gidx1", [P, TOT1 * 8], i16,
                              kind="ExternalInput")
    dstloc1_in = nc.dram_tensor("dstloc1", [P, NPC1], f16,
                                kind="ExternalInput")
    gidx2_in = nc.dram_tensor("gidx2", [P, TOT2 * 8], i16,
                              kind="ExternalInput")
    dstloc2_in = nc.dram_tensor("dstloc2", [P, NPC2], f16,
                                kind="ExternalInput")
    w1_in = nc.dram_tensor("w1", [DIM, HID], f16, kind="ExternalInput")
    w2_in = nc.dram_tensor("w2", [HID, DIM], f16, kind="ExternalInput")
    b1_in = nc.dram_tensor("b1", [P, 1], f32, kind="ExternalInput")
    b2b_in = nc.dram_tensor("b2b", [P, DIM], f32, kind="ExternalInput")
    iota_in = nc.dram_tensor("iotaf", [P, P], f16, kind="ExternalInput")
    ident_in = nc.dram_tensor("identf", [P, P], f16, kind="ExternalInput")
    uloc_in = nc.dram_tensor("u_loc", [P, BT], i32, kind="ExternalInput")
    ipos_in = nc.dram_tensor("i_pos", [P, BT], i32, kind="ExternalInput")
    degi_in = nc.dram_tensor("degi", [P, BT], i32, kind="ExternalInput")
    out_t = nc.dram_tensor("out", [P, BT], f32, kind="ExternalOutput")

    rg = [list(range(N_CORES))]

    groups2 = []
    t = 0
    while t < TOT2:
        ntl = min(GATHER_TILES, TOT2 - t)
        groups2.append((t, ntl))
        t += ntl
    gmax = GATHER_TILES

    def group_of(tg, groups, key):
        for g in groups:
            if key(g)[0] <= tg < key(g)[0] + key(g)[1]:
                return key(g)[0]
        raise AssertionError(tg)

    with tile.TileContext(nc) as tc:
        with (
            tc.tile_pool(name="const", bufs=1) as cpool,
            tc.tile_pool(name="prep", bufs=2) as ppool,
            tc.tile_pool(name="acc", bufs=1) as accpool,
            tc.tile_pool(name="msg", bufs=MSG_BUFS) as msgpool,
            tc.tile_pool(name="oneh", bufs=4) as spool,
            tc.tile_pool(name="flush", bufs=3) as fpool,
            tc.tile_pool(name="pag", bufs=3, space="PSUM") as pagg,
            tc.tile_pool(name="pfa", bufs=2, space="PSUM") as pfa,
            tc.tile_pool(name="pfb", bufs=2, space="PSUM") as pfb,
            tc.tile_pool(name="dram", bufs=1, space="DRAM") as dpool,
        ):
            # ---------- constants ----------
            iota_sb = cpool.tile([P, P], f16)
            nc.sync.dma_start(iota_sb[:], iota_in[:, :])
            ident_sb = cpool.tile([P, P], f16)
            nc.sync.dma_start(ident_sb[:], ident_in[:, :])
            w1_sb = cpool.tile([DIM, HID], f16)
            nc.sync.dma_start(w1_sb[:], w1_in[:, :])
            w2_sb = cpool.tile([HID, DIM], f16)
            nc.sync.dma_start(w2_sb[:], w2_in[:, :])
            b1_sb = cpool.tile([P, 1], f32)
            nc.sync.dma_start(b1_sb[:], b1_in[:, :])
            b2b_sb = cpool.tile([P, DIM], f32)
            nc.sync.dma_start(b2b_sb[:], b2b_in[:, :])

            gidx1_sb = cpool.tile([P, TOT1 * 8], i16)
            nc.sync.dma_start(gidx1_sb[:], gidx1_in[:, :])
            dstloc1_sb = cpool.tile([P, NPC1], f16)
            nc.sync.dma_start(dstloc1_sb[:], dstloc1_in[:, :])
            gidx2_sb = cpool.tile([P, TOT2 * 8], i16)
            nc.sync.dma_start(gidx2_sb[:], gidx2_in[:, :])
            dstloc2_sb = cpool.tile([P, NPC2], f16)
            nc.sync.dma_start(dstloc2_sb[:], dstloc2_in[:, :])

            uloc_sb = cpool.tile([P, BT], i32)
            nc.sync.dma_start(uloc_sb[:], uloc_in[:, :])
            ipos_sb = cpool.tile([P, BT], i32)
            nc.sync.dma_start(ipos_sb[:], ipos_in[:, :])
            degi_sb = cpool.tile([P, BT], i32)
            nc.sync.dma_start(degi_sb[:], degi_in[:, :])

            # ---------- dinv ----------
            deg_sb = cpool.tile([P, NB], i32)
            nc.sync.dma_start(deg_sb[:], deg_in[:, :])
            degf = cpool.tile([P, NB], f32)
            nc.vector.tensor_copy(degf[:], deg_sb[:])
            dsq = cpool.tile([P, NB], f32)
            nc.scalar.activation(dsq[:], degf[:], AF.Sqrt, bias=1.0, scale=1.0)
            dinv_sb = cpool.tile([P, NB], f32)
            nc.vector.reciprocal(dinv_sb[:], dsq[:])

            # ---------- final-phase user gathers (early: Q7 idle now) ----
            us = cpool.tile([P, BT * DIM], f32, tag="us")
            for t in range(BT):
                nc.gpsimd.indirect_dma_start(
                    out=us[:, t * DIM:(t + 1) * DIM],
                    out_offset=None, in_=users_part[:, :],
                    in_offset=bass.IndirectOffsetOnAxis(
                        ap=uloc_sb[:, t:t + 1], axis=0))

            # ---------- xs slice ----------
            acc = accpool.tile([P, NB * DIM], f16, tag="acc")
            for pp in range(PREP_PASSES):
                s0 = pp * PREP_SLOTS
                ns = PREP_SLOTS
                item_sb = ppool.tile([P, ns * DIM], f32, tag="item")
                nc.sync.dma_start(
                    item_sb[:].rearrange("p (j f) -> p j f", f=DIM),
                    item_part[:, :].rearrange("(j p) f -> p j f", p=P)
                    [:, s0:s0 + ns, :])
                sq = ppool.tile([P, ns * DIM], f32, tag="sq")
                nc.vector.tensor_tensor(sq[:], item_sb[:], item_sb[:],
                                        ALU.mult)
                ss = ppool.tile([P, ns], f32, tag="ss")
                nc.vector.tensor_reduce(
                    ss[:], sq[:].rearrange("p (j f) -> p j f", f=DIM),
                    axis=mybir.AxisListType.X, op=ALU.add)
                nrm = ppool.tile([P, ns], f32, tag="nrm")
                nc.scalar.activation(nrm[:], ss[:], AF.Sqrt)
                nrm1 = ppool.tile([P, ns], f32, tag="nrm1")
                nc.vector.tensor_scalar_max(nrm1[:], nrm[:], 1.0)
                rinv = ppool.tile([P, ns], f32, tag="rinv")
                nc.vector.reciprocal(rinv[:], nrm1[:])
                scal = ppool.tile([P, ns], f32, tag="scal")
                nc.vector.tensor_tensor(scal[:], rinv[:],
                                        dinv_sb[:, s0:s0 + ns], ALU.mult)
                # 2D first-column op absorbs waits; 3D op needs <=1 wait
                nc.vector.tensor_tensor(acc[:, s0 * DIM:s0 * DIM + 1],
                                        item_sb[:, 0:1], scal[:, 0:1],
                                        ALU.mult)
                nc.vector.tensor_tensor(
                    acc[:, s0 * DIM:(s0 + ns) * DIM].rearrange(
                        "p (j f) -> p j f", f=DIM),
                    item_sb[:].rearrange("p (j f) -> p j f", f=DIM),
                    scal[:, :, None].to_broadcast([P, ns, DIM]),
                    ALU.mult)

            xs_slice = dpool.tile([NODES_PER_CORE, ROWP], f16)
            nc.sync.dma_start(
                xs_slice[:].rearrange("(j p) F -> p j F", p=P)[:, :, 0:DIM],
                acc[:].rearrange("p (j f) -> p j f", f=DIM))
            xs_full = dpool.tile([NODES_PAD, ROWP], f16, addr_space="Shared")
            nc.gpsimd.collective_compute(
                "AllGather", ALU.bypass, replica_groups=rg,
                ins=[xs_slice.opt()], outs=[xs_full.opt()])

            # self-loop term: aggall starts at xs (f16 -> f32 copy)
            aggall = accpool.tile([P, NB * DIM], f32, tag="aggall")
            nc.vector.tensor_copy(aggall[:], acc[:])

            # final-phase user-side math (overlaps the layer-1 grind)
            ub2 = cpool.tile([P, BT * DIM], f32, tag="ub2")
            nc.vector.tensor_tensor(
                ub2[:].rearrange("p (t f) -> p t f", f=DIM),
                us[:].rearrange("p (t f) -> p t f", f=DIM),
                b2b_sb[:, None, :].to_broadcast([P, BT, DIM]), ALU.mult)
            db2 = cpool.tile([P, BT], f32)
            nc.vector.tensor_reduce(
                db2[:], ub2[:].rearrange("p (t f) -> p t f", f=DIM),
                axis=mybir.AxisListType.X, op=ALU.add)
            usq = cpool.tile([P, BT * DIM], f32, tag="usq")
            nc.vector.tensor_tensor(usq[:], us[:], us[:], ALU.mult)
            ssu = cpool.tile([P, BT], f32)
            nc.vector.tensor_reduce(
                ssu[:], usq[:].rearrange("p (t f) -> p t f", f=DIM),
                axis=mybir.AxisListType.X, op=ALU.add)
            un = cpool.tile([P, BT], f32)
            nc.scalar.activation(un[:], ssu[:], AF.Sqrt)
            un1 = cpool.tile([P, BT], f32)
            nc.vector.tensor_scalar_max(un1[:], un[:], 1.0)
            ur = cpool.tile([P, BT], f32)
            nc.vector.reciprocal(ur[:], un1[:])
            degif = cpool.tile([P, BT], f32)
            nc.vector.tensor_copy(degif[:], degi_sb[:])
            dgs = cpool.tile([P, BT], f32)
            nc.scalar.activation(dgs[:], degif[:], AF.Sqrt, bias=1.0,
                                 scale=1.0)
            dinvi = cpool.tile([P, BT], f32)
            nc.vector.reciprocal(dinvi[:], dgs[:])

            # ---------- layer 1: window-interleaved gather/agg/flush ----
            cells1 = {}
            for (c, s, tg0, col0, npc) in plist1:
                cells1[(c, s)] = (tg0, col0, npc)

            def flush_slot(s):
                sl = slice(s * DIM, (s + 1) * DIM)
                aggsb = fpool.tile([P, DIM], f16, tag="aggsb")
                nc.scalar.activation(aggsb[:], aggall[:, sl], AF.Copy,
                                     scale=dinv_sb[:, s:s + 1])
                aggT = pfa.tile([DIM, P], f16, tag="pfa")
                nc.tensor.transpose(aggT[:], aggsb[:], ident_sb[:])
                aggT_sb = fpool.tile([DIM, P], f16, tag="aggT")
                nc.scalar.activation(aggT_sb[:], aggT[:], AF.Copy)
                U = pfb.tile([P, P], f32, tag="pfb")
                nc.tensor.matmul(U[:], lhsT=w1_sb[:], rhs=aggT_sb[:],
                                 start=True, stop=True)
                H = fpool.tile([P, P], f16, tag="H")
                nc.scalar.activation(H[:], U[:], AF.Relu,
                                     bias=b1_sb[:, 0:1], scale=1.0)
                z2p = pfa.tile([DIM, P], f32, tag="pfa")
                nc.tensor.matmul(z2p[:], lhsT=w2_sb[:], rhs=H[:],
                                 start=True, stop=True)
                z2p_sb = fpool.tile([DIM, P], f16, tag="z2p")
                nc.scalar.activation(z2p_sb[:], z2p[:], AF.Copy)
                z2r = pfb.tile([P, DIM], f16, tag="pfb")
                nc.tensor.transpose(z2r[:], z2p_sb[:],
                                    ident_sb[:DIM, :DIM])
                nc.scalar.activation(acc[:, sl], z2r[:], AF.Copy,
                                     scale=dinv_sb[:, s:s + 1])

            z2_slice = dpool.tile([NODES_PER_CORE, ROWP], f16)
            msgs1 = {}
            tile2grp = {}
            WS = WINDOW_SLOTS
            for w0 in range(0, NB, WS):
                ws = range(w0, min(w0 + WS, NB))
                # issue gathers chunk-major (boundary tiles re-gathered)
                for c in range(NCHUNK):
                    lo, hi = None, -1
                    for s in ws:
                        if (c, s) in cells1:
                            tg0, _, npc = cells1[(c, s)]
                            lo = tg0 if lo is None else min(lo, tg0)
                            hi = max(hi, tg0 + npc - 1)
                    if lo is None:
                        continue
                    t = lo
                    crows = min(CHUNK, NODES_PAD - c * CHUNK)
                    while t <= hi:
                        ntl = min(GATHER_TILES, hi + 1 - t)
                        m = msgpool.tile([P, gmax * ROWP], f16, tag="msg")
                        nc.gpsimd.dma_gather(
                            m[:, :ntl * ROWP].rearrange(
                                "p (t F) -> p t F", F=ROWP),
                            xs_full[c * CHUNK:c * CHUNK + crows, :],
                            gidx1_sb[:, t * 8:(t + ntl) * 8],
                            ntl * P, ntl * P, ROWP, single_packet=False)
                        msgs1[t] = m
                        for tt in range(t, t + ntl):
                            tile2grp[tt] = t
                        t += ntl
                # aggregate chunk-major (must match gather issue order)
                for c in range(NCHUNK):
                    for s in ws:
                        if (c, s) not in cells1:
                            continue
                        tg0, col0, npc = cells1[(c, s)]
                        agg = pagg.tile([P, DIM], f32, tag="agg")
                        for pi in range(npc):
                            tg = tg0 + pi
                            g0 = tile2grp[tg]
                            m = msgs1[g0]
                            tl = tg - g0
                            Sx = spool.tile([P, P], f16, tag="oneh")
                            nc.vector.tensor_tensor(
                                Sx[:],
                                dstloc1_sb[:, col0 + pi:col0 + pi + 1]
                                .to_broadcast([P, P]),
                                iota_sb[:], ALU.is_equal)
                            nc.tensor.matmul(
                                agg[:], lhsT=Sx[:],
                                rhs=m[:].rearrange("p (t F) -> p t F",
                                                   F=ROWP)[:, tl, 0:DIM],
                                start=(pi == 0), stop=(pi == npc - 1))
                        sl = slice(s * DIM, (s + 1) * DIM)
                        nc.vector.tensor_tensor(aggall[:, sl], aggall[:, sl],
                                                agg[:], ALU.add)
                # flush + z2 window write (hides under later windows' Q7)
                for s in ws:
                    flush_slot(s)
                nw = len(ws)
                nc.sync.dma_start(
                    z2_slice[:].rearrange("(j p) F -> p j F", p=P)
                    [:, w0:w0 + nw, 0:DIM],
                    acc[:, w0 * DIM:(w0 + nw) * DIM]
                    .rearrange("p (j f) -> p j f", f=DIM))

            # ---------- layer 2: gathers from local z2 ----------
            msgs2 = {}
            for (t0, ntl) in groups2:
                m = msgpool.tile([P, gmax * ROWP], f16, tag="msg")
                nc.gpsimd.dma_gather(
                    m[:, :ntl * ROWP].rearrange("p (t F) -> p t F", F=ROWP),
                    z2_slice[:, :],
                    gidx2_sb[:, t0 * 8:(t0 + ntl) * 8],
                    ntl * P, ntl * P, ROWP, single_packet=False)
                msgs2[t0] = m

            # ---------- layer 2: piece aggregation into compact ----------
            compact_sb = accpool.tile([P, NCB * DIM], f16, tag="compact")
            for (s, t0c, col0, npc) in plist2:
                agg = pagg.tile([P, DIM], f32, tag="agg")
                for pi in range(npc):
                    tg = t0c + pi
                    g0 = group_of(tg, groups2, lambda g: (g[0], g[1]))
                    m = msgs2[g0]
                    tl = tg - g0
                    Sx = spool.tile([P, P], f16, tag="oneh")
                    nc.vector.tensor_tensor(
                        Sx[:],
                        dstloc2_sb[:, col0 + pi:col0 + pi + 1]
                        .to_broadcast([P, P]),
                        iota_sb[:], ALU.is_equal)
                    nc.tensor.matmul(
                        agg[:], lhsT=Sx[:],
                        rhs=m[:].rearrange("p (t F) -> p t F",
                                           F=ROWP)[:, tl, 0:DIM],
                        start=(pi == 0), stop=(pi == npc - 1))
                nc.scalar.activation(compact_sb[:, s * DIM:(s + 1) * DIM],
                                     agg[:], AF.Copy)

            # ---------- compact partials -> AllReduce (f16) ----------
            part_c = dpool.tile([NCB * P, DIM], f16)
            nc.sync.dma_start(
                part_c[:].rearrange("(j p) f -> p j f", p=P),
                compact_sb[:].rearrange("p (j f) -> p j f", f=DIM))
            ar_c = dpool.tile([NCB * P, DIM], f16, addr_space="Shared")
            nc.gpsimd.collective_compute(
                "AllReduce", ALU.add, replica_groups=rg,
                ins=[part_c.opt()], outs=[ar_c.opt()])

            # ---------- final: item gather + dot + sigmoid ----------
            it = cpool.tile([P, BT * DIM], f16, tag="it")
            for t in range(BT):
                nc.gpsimd.indirect_dma_start(
                    out=it[:, t * DIM:(t + 1) * DIM],
                    out_offset=None, in_=ar_c[:],
                    in_offset=bass.IndirectOffsetOnAxis(
                        ap=ipos_sb[:, t:t + 1], axis=0))
            it32 = cpool.tile([P, BT * DIM], f32, tag="it32")
            nc.vector.tensor_copy(it32[:], it[:])

            pr = cpool.tile([P, BT * DIM], f32, tag="pr")
            nc.vector.tensor_tensor(pr[:], us[:], it32[:], ALU.mult)
            d = cpool.tile([P, BT], f32)
            nc.vector.tensor_reduce(
                d[:], pr[:].rearrange("p (t f) -> p t f", f=DIM),
                axis=mybir.AxisListType.X, op=ALU.add)

            dsc = cpool.tile([P, BT], f32)
            nc.vector.tensor_tensor(dsc[:], d[:], dinvi[:], ALU.mult)
            dtot = cpool.tile([P, BT], f32)
            nc.vector.tensor_tensor(dtot[:], dsc[:], db2[:], ALU.add)

            d2 = cpool.tile([P, BT], f32)
            nc.vector.tensor_tensor(d2[:], dtot[:], ur[:], ALU.mult)
            sg = cpool.tile([P, BT], f32)
            nc.scalar.activation(sg[:], d2[:], AF.Sigmoid)
            nc.sync.dma_start(out_t[:, :], sg[:])

    nc.compile()
    return nc


def _legalize_waits(nc):
    """Split multi-sem waits off compute instructions onto preceding NoOps.

    walrus codegen's per-instruction ISA structs have a single sync-wait
    slot; Tile can attach several.  Standalone NoOps carry the extras (one
    wait per NoOp, executed in engine program order just before the
    instruction, so semantics are unchanged).
    """
    keep = (mybir.InstCollectiveCompute, mybir.InstNoOp,
            mybir.InstEventSemaphore, mybir.InstAllEngineBarrier)
    fn = nc.m.functions[0]
    n_split = 0
    for blk in fn.blocks:
        insts = list(blk.instructions)
        out = []
        changed = False
        for inst in insts:
            si = inst.sync_info
            if (si is not None and len(si.on_wait) > 1
                    and not isinstance(inst, keep)):
                waits = list(si.on_wait)
                for w_i, w in enumerate(waits[:-1]):
                    nop = mybir.InstNoOp(
                        name=f"wnop{w_i}_{inst.name}", ins=[], outs=[])
                    nop.engine = inst.engine
                    nop.sync_info = mybir.SyncInfo(on_wait=[w], on_update=[])
                    out.append(nop)
                inst.sync_info = mybir.SyncInfo(
                    on_wait=[waits[-1]], on_update=list(si.on_update))
                changed = True
                n_split += 1
            out.append(inst)
        if changed:
            blk.instructions = out
    return n_split


# ---------------------------------------------------------------- runner
_BUILD_CACHE = {}


def _get_bass(struct):
    key = (struct["T1"], struct["TOT1"], struct["NPC1"], struct["plist1"],
           struct["TOT2"], struct["NPC2"], struct["plist2"],
           struct["NCB"], struct["BT"])
    if key not in _BUILD_CACHE:
        nc = _build_bass(struct)
        _legalize_waits(nc)
        _BUILD_CACHE[key] = nc
    return _BUILD_CACHE[key]


def _make_in_maps(L, item_emb, users_emb, W1, b1, W2, b2):
    item_pad = np.zeros((NODES_PAD, DIM), np.float32)
    item_pad[:N_ITEMS] = np.asarray(item_emb, np.float32)
    item_perm = item_pad[L["node_of_pos"]]

    w1h = np.asarray(W1, np.float32).astype(np.float16)
    w2h = np.asarray(W2, np.float32).astype(np.float16)
    b1c = np.asarray(b1, np.float32).reshape(HID, 1)
    b2b = np.broadcast_to(np.asarray(b2, np.float32), (P, DIM)).copy()
    iotaf = np.broadcast_to(np.arange(P, dtype=np.float16), (P, P)).copy()
    identf = np.eye(P, dtype=np.float16)
    users = np.asarray(users_emb, np.float32)

    in_maps = []
    for k in range(N_CORES):
        in_maps.append({
            "item_part": item_perm[k * NODES_PER_CORE:(k + 1) * NODES_PER_CORE],
            "users_part": users[k * USERS_PER_CORE:(k + 1) * USERS_PER_CORE],
            "deg": L["deg_cs"][k],
            "gidx1": L["gidx1"][k], "dstloc1": L["dstloc1"][k],
            "gidx2": L["gidx2"][k], "dstloc2": L["dstloc2"][k],
            "w1": w1h, "w2": w2h, "b1": b1c, "b2b": b2b,
            "iotaf": iotaf, "identf": identf,
            "u_loc": L["u_loc"][k], "i_pos": L["i_pos"][k],
            "degi": L["degi"][k],
        })
    return in_maps


def _ensure_ntff_hook():
    """Register the axon NTFF profiling hook if the image lacks
    antenv.axon_hooks (needed for trace=True under axon)."""
    import sys
    import types
    try:
        from antenv.axon_hooks import get_axon_ntff_profile_hook  # noqa: F401
        return
    except ImportError:
        pass
    try:
        import antenv
        from trn_agent_boot.trn_boot import _ntff_profile_via_ctypes
        hook = _ntff_profile_via_ctypes("/opt/axon/libaxon_pjrt.so")
        mod = types.ModuleType("antenv.axon_hooks")
        box = {"v": hook}
        mod.set_axon_ntff_profile_hook = lambda h: box.__setitem__("v", h)
        mod.get_axon_ntff_profile_hook = lambda: box["v"]
        sys.modules["antenv.axon_hooks"] = mod
        antenv.axon_hooks = mod
    except Exception:
        pass


def run(u, i, edges, users_emb, item_emb, W1, b1, W2, b2, trace=False):
    if trace:
        _ensure_ntff_hook()
    L = _build_layout(u, i, edges)
    nc = _get_bass(L["struct"])
    in_maps = _make_in_maps(L, item_emb, users_emb, W1, b1, W2, b2)
    res = run_bass_kernel_spmd(nc, in_maps, core_ids=list(range(N_CORES)),
                               trace=trace)
    logits = np.zeros(B_PAIRS, np.float32)
    for k in range(N_CORES):
        sel = L["perm"][k]
        o = np.asarray(res.results[k]["out"])
        logits[sel] = o.T.ravel()[:len(sel)]
    return logits, res


def kernel(u, i, edges, users_emb, item_emb, W1, b1, W2, b2):
    logits, _ = run(u, i, edges, users_emb, item_emb, W1, b1, W2, b2,
                    trace=bool(int(os.environ.get("GCN_TRACE", "0"))))
    return logits
